# revision 10
# baseline (speedup 1.0000x reference)
"""Multi-head attention (RoPE) Trainium2 kernel, 8 NeuronCores.

Sharding: data-parallel over batch (2) x tensor-parallel over heads (4
heads/core).  Core c handles batch c//4, heads 4*(c%4) .. 4*(c%4)+4.
Each core computes qkv projection for its heads, RoPE, full attention
over its heads, and the out-projection partial (w_out column shard).
The 4 partials per batch are summed on the host (TP all-reduce epilogue
done host-side; no device collective).

Device layouts (per core):
  xt    [128, DIM/128, S]    bf16   x[b].T tiled: xt[p, kt, s] = x[b, s, kt*128+p]
  wq/wk [r, 128, DIM/128, 128] bf16 stationary tiles for transposed proj
  wv    [128, DIM/128, HL*128] bf16 moving tiles for natural v proj
  wo    [128, HL, DIM]       bf16   wo[p, h, o] = w_out[o, 512g + h*128 + p]
  cosf/sinfs [128, S]        bf16   RoPE tables, transposed, halves duplicated,
                                    sin sign-folded (rows 0:64 negated)
  out   [S, DIM]             f32    partial output (natural layout)
"""

import math
import sys
from contextlib import ExitStack

import numpy as np

sys.path.insert(0, "/opt/trn_rl_repo")

import ml_dtypes  # noqa: E402

import concourse.bass as bass  # noqa: E402
import concourse.tile as tile  # noqa: E402
from concourse import bacc, mybir  # noqa: E402
from concourse.bass_utils import run_bass_kernel_spmd  # noqa: E402

P = 128          # partitions / head dim
HEADS = 16
D = 128
N_CORES = 8
DP = 2           # batch shards
TP = 4           # head-group shards
HL = HEADS // TP  # heads per core

BF16 = mybir.dt.bfloat16
F32 = mybir.dt.float32


def build_kernel(S, DIM, HL_=HL, sq=512, sq2=1024, num_devices=N_CORES,
                 debug_taps=False):
    """Build + compile the per-core Bass program (SPMD: same program all cores)."""
    nkt = DIM // P    # contraction tiles for projections
    nst = S // P      # sequence tiles of 128
    nsq = S // sq     # 512-wide free blocks
    sq2 = min(sq2, S)
    nsq2 = S // sq2   # attention s_q groups
    sq_per2 = sq2 // sq
    scale = 1.0 / math.sqrt(D)

    nc = bacc.Bacc("TRN2", debug=False, num_devices=num_devices)

    xt = nc.dram_tensor("xt", [P, nkt, S], BF16, kind="ExternalInput").ap()
    wq = nc.dram_tensor("wq", [HL_, P, nkt, P], BF16, kind="ExternalInput").ap()
    wk = nc.dram_tensor("wk", [HL_, P, nkt, P], BF16, kind="ExternalInput").ap()
    wv = nc.dram_tensor("wv", [P, nkt, HL_ * D], BF16, kind="ExternalInput").ap()
    wo = nc.dram_tensor("wo", [P, HL_, DIM], BF16, kind="ExternalInput").ap()
    cosf = nc.dram_tensor("cosf", [P, S], BF16, kind="ExternalInput").ap()
    sinfs = nc.dram_tensor("sinfs", [P, S], BF16, kind="ExternalInput").ap()
    out = nc.dram_tensor("out", [S, DIM], F32, kind="ExternalOutput").ap()
    out_t = out.rearrange("(st p) o -> st p o", p=P)
    taps = {}
    if debug_taps:
        for name, shape in (
            ("dbg_q", [P, HL_, S]), ("dbg_k", [P, HL_, S]),
            ("dbg_v", [P, S // P, HL_ * D]), ("dbg_attn", [P, HL_, S]),
        ):
            taps[name] = nc.dram_tensor(
                name, shape, BF16, kind="ExternalOutput").ap()
        for name, shape in (
            ("dbg_ex", [P, sq2]), ("dbg_sm", [1, sq2]),
            ("dbg_av", [P, sq2]), ("dbg_bc", [P, sq2]),
        ):
            taps[name] = nc.dram_tensor(
                name, shape, F32, kind="ExternalOutput").ap()

    with tile.TileContext(nc) as tc, ExitStack() as ctx:
        nc = tc.nc
        # ---- persistent SBUF ----
        big = ctx.enter_context(tc.tile_pool(name="big", bufs=1))
        q_sb = big.tile([P, HL_, S], BF16, tag="q")
        k_sb = big.tile([P, HL_, S], BF16, tag="k")
        v_sb = big.tile([P, nst, HL_ * D], BF16, tag="v")
        attn_sb = big.tile([P, HL_, S], BF16, tag="attn")
        cos_sb = big.tile([P, S], BF16, tag="cos")
        sin_sb = big.tile([P, S], BF16, tag="sin")
        ones_sb = big.tile([P, 1], BF16, tag="ones")
        wv_sb = big.tile([P, nkt, HL_ * D], BF16, tag="wv")
        wo_sb = big.tile([P, HL_, DIM], BF16, tag="wo")

        nc.vector.memset(ones_sb, 1.0)
        nc.sync.dma_start(out=cos_sb, in_=cosf)
        nc.sync.dma_start(out=sin_sb, in_=sinfs)
        nc.sync.dma_start(out=wv_sb, in_=wv)
        nc.sync.dma_start(out=wo_sb, in_=wo)

        xpool = ctx.enter_context(tc.tile_pool(name="xpool", bufs=2))
        wpool = ctx.enter_context(tc.tile_pool(name="wpool", bufs=3))
        rope = ctx.enter_context(tc.tile_pool(name="rope", bufs=2))

        # ================= Phase 1: qkv projection + RoPE =================
        # x streamed in sq-wide chunks; all 12 output row-tiles per chunk.
        spt = sq // P  # s-tiles of 128 per chunk
        with tc.tile_pool(name="psA", bufs=4, space="PSUM") as psA:
            for j in range(nsq):
                win = bass.ds(j * sq, sq)
                xc = xpool.tile([P, nkt, sq], BF16, tag="xc")
                for kt in range(nkt):
                    nc.sync.dma_start(out=xc[:, kt, :], in_=xt[:, kt, win])
                # q and k (transposed orientation [d, s])
                for which, wdram, dst in (("q", wq, q_sb), ("k", wk, k_sb)):
                    for h in range(HL_):
                        w_t = wpool.tile([P, nkt, P], BF16, tag="w")
                        nc.sync.dma_start(out=w_t, in_=wdram[h])
                        ps = psA.tile([P, sq], F32, tag="ps")
                        for kt in range(nkt):
                            nc.tensor.matmul(
                                ps,
                                lhsT=w_t[:, kt, :],
                                rhs=xc[:, kt, :],
                                start=(kt == 0),
                                stop=(kt == nkt - 1),
                            )
                        # RoPE: dst = p*cos + swap(p)*sin_signed
                        pb = rope.tile([P, sq], BF16, tag="pb")
                        nc.scalar.copy(pb, ps)
                        sw = rope.tile([P, sq], BF16, tag="sw")
                        nc.vector.tensor_copy(out=sw[0:64, :], in_=pb[64:128, :])
                        nc.vector.tensor_copy(out=sw[64:128, :], in_=pb[0:64, :])
                        t1 = rope.tile([P, sq], BF16, tag="t1")
                        nc.vector.tensor_mul(t1, pb, cos_sb[:, win])
                        t2 = rope.tile([P, sq], BF16, tag="t2")
                        nc.vector.tensor_mul(t2, sw, sin_sb[:, win])
                        nc.vector.tensor_add(dst[:, h, win], t1, t2)
                # v (natural orientation [s, d_local])
                for sl in range(spt):
                    st = j * spt + sl
                    ps = psA.tile([P, HL_ * D], F32, tag="ps")
                    for kt in range(nkt):
                        nc.tensor.matmul(
                            ps,
                            lhsT=xc[:, kt, sl * P:(sl + 1) * P],
                            rhs=wv_sb[:, kt, :],
                            start=(kt == 0),
                            stop=(kt == nkt - 1),
                        )
                    nc.scalar.copy(v_sb[:, st, :], ps)

        # ================= Phase 2: attention =================
        expp = ctx.enter_context(tc.tile_pool(name="expp", bufs=3))
        nrm = ctx.enter_context(tc.tile_pool(name="nrm", bufs=1))
        with (
            tc.tile_pool(name="psLG", bufs=2, space="PSUM") as psLG,
            tc.tile_pool(name="psAV", bufs=1, space="PSUM") as psAV,
            tc.tile_pool(name="psSM", bufs=1, space="PSUM") as psSM,
        ):
            for h in range(HL_):
                for j2 in range(nsq2):
                    win2 = bass.ds(j2 * sq2, sq2)
                    av = psAV.tile([P, sq2], F32, tag="av")
                    sm = psSM.tile([1, sq2], F32, tag="sm")
                    for i in range(nst):
                        lg = psLG.tile([P, sq2], F32, tag="lg")
                        k_tile = k_sb[:, h, i * P:(i + 1) * P]
                        for c in range(sq_per2):
                            nc.tensor.matmul(
                                lg[:, c * sq:(c + 1) * sq],
                                lhsT=k_tile,
                                rhs=q_sb[:, h, bass.ds(j2 * sq2 + c * sq, sq)],
                                start=True,
                                stop=True,
                            )
                        ex = expp.tile([P, sq2], BF16, tag="ex")
                        nc.scalar.activation(
                            ex, lg, mybir.ActivationFunctionType.Exp, scale=scale
                        )
                        if debug_taps and h == 0 and j2 == 0 and i == 0:
                            exf = nrm.tile([P, sq2], F32, tag="dbgex")
                            nc.vector.tensor_copy(out=exf, in_=ex)
                            nc.sync.dma_start(out=taps["dbg_ex"], in_=exf)
                        v_tile = v_sb[:, i, h * D:(h + 1) * D]
                        for c in range(sq_per2):
                            cw = bass.ds(c * sq, sq)
                            nc.tensor.matmul(
                                sm[:, cw],
                                lhsT=ones_sb,
                                rhs=ex[:, cw],
                                start=(i == 0),
                                stop=(i == nst - 1),
                            )
                            nc.tensor.matmul(
                                av[:, cw],
                                lhsT=v_tile,
                                rhs=ex[:, cw],
                                start=(i == 0),
                                stop=(i == nst - 1),
                            )
                    recip = nrm.tile([1, sq2], F32, tag="recip")
                    nc.vector.reciprocal(recip, sm)
                    bcast = nrm.tile([P, sq2], F32, tag="bcast")
                    nc.gpsimd.partition_broadcast(bcast, recip)
                    if debug_taps and h == 0 and j2 == 0:
                        smf = nrm.tile([1, sq2], F32, tag="dbgsm")
                        nc.vector.tensor_copy(out=smf, in_=sm)
                        nc.sync.dma_start(out=taps["dbg_sm"], in_=smf)
                        avf = nrm.tile([P, sq2], F32, tag="dbgav")
                        nc.vector.tensor_copy(out=avf, in_=av)
                        nc.sync.dma_start(out=taps["dbg_av"], in_=avf)
                        nc.sync.dma_start(out=taps["dbg_bc"], in_=bcast)
                    nc.vector.tensor_mul(attn_sb[:, h, win2], av, bcast)

        if debug_taps:
            nc.sync.dma_start(out=taps["dbg_q"], in_=q_sb)
            nc.sync.dma_start(out=taps["dbg_k"], in_=k_sb)
            nc.sync.dma_start(out=taps["dbg_v"], in_=v_sb)
            nc.sync.dma_start(out=taps["dbg_attn"], in_=attn_sb)

        # ================= Phase 3: out-projection =================
        outp = ctx.enter_context(tc.tile_pool(name="outp", bufs=3))
        nob = DIM // sq
        with tc.tile_pool(name="psO", bufs=4, space="PSUM") as psO:
            for st in range(nst):
                for ob in range(nob):
                    po = psO.tile([P, sq], F32, tag="po")
                    for h in range(HL_):
                        nc.tensor.matmul(
                            po,
                            lhsT=attn_sb[:, h, st * P:(st + 1) * P],
                            rhs=wo_sb[:, h, ob * sq:(ob + 1) * sq],
                            start=(h == 0),
                            stop=(h == HL_ - 1),
                        )
                    ot = outp.tile([P, sq], F32, tag="ot")
                    if (st * nob + ob) % 2 == 0:
                        nc.scalar.copy(ot, po)
                    else:
                        nc.vector.tensor_copy(out=ot, in_=po)
                    nc.sync.dma_start(
                        out=out_t[st][:, ob * sq:(ob + 1) * sq], in_=ot
                    )

    nc.compile()
    return nc


# ---------------- host side ----------------

def _rope_tables(S):
    inv_freq = 1.0 / (10000.0 ** (np.arange(0, D, 2, dtype=np.float32) / D))
    t = np.arange(S, dtype=np.float32)
    freqs = np.einsum("i,j->ij", t, inv_freq)      # [S, 64]
    cos_h = np.cos(freqs).T                        # [64, S]
    sin_h = np.sin(freqs).T
    cosf = np.concatenate([cos_h, cos_h], 0)       # [128, S]
    sinfs = np.concatenate([-sin_h, sin_h], 0)     # sign-folded
    return cosf, sinfs


def _shard_inputs(x, w_qkv, w_out):
    B, S, DIM = x.shape
    bf = ml_dtypes.bfloat16
    nkt = DIM // P
    cosf, sinfs = _rope_tables(S)
    cosf = cosf.astype(bf)
    sinfs = sinfs.astype(bf)
    in_maps = []
    for c in range(N_CORES):
        b, g = divmod(c, TP)
        h0 = HL * g
        # xt[p, kt, s] = x[b, s, kt*128+p]
        xt = np.ascontiguousarray(
            x[b].reshape(S, nkt, P).transpose(2, 1, 0)
        ).astype(bf)
        # wq[r, p, kt, m] = w_qkv[(h0+r)*128 + m, kt*128 + p]
        wq_s = w_qkv[h0 * D:(h0 + HL) * D]                  # [512, DIM]
        wk_s = w_qkv[HEADS * D + h0 * D:HEADS * D + (h0 + HL) * D]
        wv_s = w_qkv[2 * HEADS * D + h0 * D:2 * HEADS * D + (h0 + HL) * D]
        wq_t = np.ascontiguousarray(
            wq_s.reshape(HL, P, nkt, P).transpose(0, 3, 2, 1)
        ).astype(bf)
        wk_t = np.ascontiguousarray(
            wk_s.reshape(HL, P, nkt, P).transpose(0, 3, 2, 1)
        ).astype(bf)
        # wv[p, kt, vo] = wv_s[vo, kt*128+p]
        wv_t = np.ascontiguousarray(
            wv_s.reshape(HL * D, nkt, P).transpose(2, 1, 0)
        ).astype(bf)
        # wo[p, h, o] = w_out[o, 512g + h*128 + p]
        wo_s = w_out[:, h0 * D:(h0 + HL) * D]               # [DIM, 512]
        wo_t = np.ascontiguousarray(
            wo_s.reshape(DIM, HL, P).transpose(2, 1, 0)
        ).astype(bf)
        in_maps.append(
            {"xt": xt, "wq": wq_t, "wk": wk_t, "wv": wv_t, "wo": wo_t,
             "cosf": cosf, "sinfs": sinfs}
        )
    return in_maps


_NC_CACHE = {}


def _get_nc(S, DIM):
    key = (S, DIM)
    if key not in _NC_CACHE:
        _NC_CACHE[key] = build_kernel(S, DIM)
    return _NC_CACHE[key]


def kernel(x, w_qkv, w_out, trace=False):
    x = np.asarray(x)
    w_qkv = np.asarray(w_qkv)
    w_out = np.asarray(w_out)
    B, S, DIM = x.shape
    nc = _get_nc(S, DIM)
    in_maps = _shard_inputs(x, w_qkv, w_out)
    res = run_bass_kernel_spmd(nc, in_maps, core_ids=list(range(N_CORES)),
                               trace=trace)
    outs = [np.asarray(r["out"], dtype=np.float32) for r in res.results]
    full = np.stack(
        [sum(outs[b * TP:(b + 1) * TP][1:], outs[b * TP]) for b in range(DP)]
    ).astype(np.float32)
    if trace:
        kernel.last_results = res
    return full


# revision 13
# speedup vs baseline: 1.0568x; 1.0568x over previous
"""Multi-head attention (RoPE) Trainium2 kernel, 8 NeuronCores.

Sharding: data-parallel over batch (2) x tensor-parallel over heads (4
heads/core).  Core c handles batch c//4, heads 4*(c%4) .. 4*(c%4)+4.
Each core computes qkv projection for its heads, RoPE, full attention
over its heads, and the out-projection partial (w_out column shard).
The 4 partials per batch are summed on the host (TP all-reduce epilogue
done host-side; no device collective).

Device layouts (per core):
  xt    [128, DIM/128, S]    bf16   x[b].T tiled: xt[p, kt, s] = x[b, s, kt*128+p]
  wq/wk [r, 128, DIM/128, 128] bf16 stationary tiles for transposed proj
  wv    [128, DIM/128, HL*128] bf16 moving tiles for natural v proj
  wo    [128, HL, DIM]       bf16   wo[p, h, o] = w_out[o, 512g + h*128 + p]
  cosf/sinfs [128, S]        bf16   RoPE tables, transposed, halves duplicated,
                                    sin sign-folded (rows 0:64 negated)
  out   [S, DIM]             f32    partial output (natural layout)
"""

import math
import sys
from contextlib import ExitStack

import numpy as np

sys.path.insert(0, "/opt/trn_rl_repo")

import ml_dtypes  # noqa: E402

import concourse.bass as bass  # noqa: E402
import concourse.tile as tile  # noqa: E402
from concourse import bacc, mybir  # noqa: E402
from concourse.bass_utils import run_bass_kernel_spmd  # noqa: E402

P = 128          # partitions / head dim
HEADS = 16
D = 128
N_CORES = 8
DP = 2           # batch shards
TP = 4           # head-group shards
HL = HEADS // TP  # heads per core

BF16 = mybir.dt.bfloat16
F32 = mybir.dt.float32


def build_kernel(S, DIM, HL_=HL, sq=512, sq2=1024, num_devices=N_CORES,
                 debug_taps=False):
    """Build + compile the per-core Bass program (SPMD: same program all cores)."""
    nkt = DIM // P    # contraction tiles for projections
    nst = S // P      # sequence tiles of 128
    nsq = S // sq     # 512-wide free blocks
    sq2 = min(sq2, S)
    nsq2 = S // sq2   # attention s_q groups
    sq_per2 = sq2 // sq
    scale = 1.0 / math.sqrt(D)

    nc = bacc.Bacc("TRN2", debug=False, num_devices=num_devices)

    xt = nc.dram_tensor("xt", [P, nkt, S], BF16, kind="ExternalInput").ap()
    wq = nc.dram_tensor("wq", [HL_, P, nkt, P], BF16, kind="ExternalInput").ap()
    wk = nc.dram_tensor("wk", [HL_, P, nkt, P], BF16, kind="ExternalInput").ap()
    wv = nc.dram_tensor("wv", [P, nkt, HL_ * D], BF16, kind="ExternalInput").ap()
    wo = nc.dram_tensor("wo", [P, HL_, DIM], BF16, kind="ExternalInput").ap()
    cosf = nc.dram_tensor("cosf", [P, S], BF16, kind="ExternalInput").ap()
    sinfs = nc.dram_tensor("sinfs", [P, S], BF16, kind="ExternalInput").ap()
    out = nc.dram_tensor("out", [S, DIM], F32, kind="ExternalOutput").ap()
    out_t = out.rearrange("(st p) o -> st p o", p=P)
    taps = {}
    if debug_taps:
        for name, shape in (
            ("dbg_q", [P, HL_, S]), ("dbg_k", [P, HL_, S]),
            ("dbg_v", [P, S // P, HL_ * D]), ("dbg_attn", [P, HL_, S]),
        ):
            taps[name] = nc.dram_tensor(
                name, shape, BF16, kind="ExternalOutput").ap()
        for name, shape in (
            ("dbg_ex", [P, sq2]), ("dbg_sm", [1, sq2]),
            ("dbg_av", [P, sq2]), ("dbg_bc", [P, sq2]),
        ):
            taps[name] = nc.dram_tensor(
                name, shape, F32, kind="ExternalOutput").ap()

    with tile.TileContext(nc) as tc, ExitStack() as ctx:
        nc = tc.nc
        # ---- persistent SBUF ----
        big = ctx.enter_context(tc.tile_pool(name="big", bufs=1))
        q_sb = big.tile([P, HL_, S], BF16, tag="q")
        k_sb = big.tile([P, HL_, S], BF16, tag="k")
        v_sb = big.tile([P, nst, HL_ * D], BF16, tag="v")
        attn_sb = big.tile([P, HL_, S], BF16, tag="attn")
        cos_sb = big.tile([P, S], BF16, tag="cos")
        sin_sb = big.tile([P, S], BF16, tag="sin")
        ones_sb = big.tile([P, 1], BF16, tag="ones")
        wv_sb = big.tile([P, nkt, HL_ * D], BF16, tag="wv")
        wo_sb = big.tile([P, HL_, DIM], BF16, tag="wo")

        nc.vector.memset(ones_sb, 1.0)
        nc.sync.dma_start(out=cos_sb, in_=cosf)
        nc.sync.dma_start(out=sin_sb, in_=sinfs)
        nc.sync.dma_start(out=wv_sb, in_=wv)
        nc.sync.dma_start(out=wo_sb, in_=wo)

        xpool = ctx.enter_context(tc.tile_pool(name="xpool", bufs=2))
        wpool = ctx.enter_context(tc.tile_pool(name="wpool", bufs=3))
        rope = ctx.enter_context(tc.tile_pool(name="rope", bufs=2))

        # ================= Phase 1: qkv projection + RoPE =================
        # x streamed in sq-wide chunks; all 12 output row-tiles per chunk.
        spt = sq // P  # s-tiles of 128 per chunk
        with tc.tile_pool(name="psA", bufs=4, space="PSUM") as psA:
            for j in range(nsq):
                win = bass.ds(j * sq, sq)
                xc = xpool.tile([P, nkt, sq], BF16, tag="xc")
                for kt in range(nkt):
                    nc.sync.dma_start(out=xc[:, kt, :], in_=xt[:, kt, win])
                # q and k (transposed orientation [d, s])
                for which, wdram, dst in (("q", wq, q_sb), ("k", wk, k_sb)):
                    for h in range(HL_):
                        w_t = wpool.tile([P, nkt, P], BF16, tag="w")
                        nc.sync.dma_start(out=w_t, in_=wdram[h])
                        ps = psA.tile([P, sq], F32, tag="ps")
                        for kt in range(nkt):
                            nc.tensor.matmul(
                                ps,
                                lhsT=w_t[:, kt, :],
                                rhs=xc[:, kt, :],
                                start=(kt == 0),
                                stop=(kt == nkt - 1),
                            )
                        # RoPE: dst = p*cos + swap(p)*sin_signed
                        pb = rope.tile([P, sq], BF16, tag="pb")
                        nc.scalar.copy(pb, ps)
                        sw = rope.tile([P, sq], BF16, tag="sw")
                        nc.vector.tensor_copy(out=sw[0:64, :], in_=pb[64:128, :])
                        nc.vector.tensor_copy(out=sw[64:128, :], in_=pb[0:64, :])
                        t1 = rope.tile([P, sq], BF16, tag="t1")
                        nc.vector.tensor_mul(t1, pb, cos_sb[:, win])
                        t2 = rope.tile([P, sq], BF16, tag="t2")
                        nc.vector.tensor_mul(t2, sw, sin_sb[:, win])
                        nc.vector.tensor_add(dst[:, h, win], t1, t2)
                # v (natural orientation [s, d_local])
                for sl in range(spt):
                    st = j * spt + sl
                    ps = psA.tile([P, HL_ * D], F32, tag="ps")
                    for kt in range(nkt):
                        nc.tensor.matmul(
                            ps,
                            lhsT=xc[:, kt, sl * P:(sl + 1) * P],
                            rhs=wv_sb[:, kt, :],
                            start=(kt == 0),
                            stop=(kt == nkt - 1),
                        )
                    nc.scalar.copy(v_sb[:, st, :], ps)

        # ================= Phase 2: attention =================
        expp = ctx.enter_context(tc.tile_pool(name="expp", bufs=3))
        nrm = ctx.enter_context(tc.tile_pool(name="nrm", bufs=1))
        with (
            tc.tile_pool(name="psLG", bufs=2, space="PSUM") as psLG,
            tc.tile_pool(name="psAV", bufs=1, space="PSUM") as psAV,
            tc.tile_pool(name="psSM", bufs=1, space="PSUM") as psSM,
        ):
            for h in range(HL_):
                for j2 in range(nsq2):
                    win2 = bass.ds(j2 * sq2, sq2)
                    av = psAV.tile([P, sq2], F32, tag="av")
                    sm = psSM.tile([1, sq2], F32, tag="sm")
                    # software-pipelined: sums/av for tile i-1 issue after
                    # logits for tile i, so PE never queues behind exp.
                    exs = [None] * nst
                    for i in range(nst):
                        lg = psLG.tile([P, sq2], F32, tag="lg")
                        k_tile = k_sb[:, h, i * P:(i + 1) * P]
                        for c in range(sq_per2):
                            nc.tensor.matmul(
                                lg[:, c * sq:(c + 1) * sq],
                                lhsT=k_tile,
                                rhs=q_sb[:, h, bass.ds(j2 * sq2 + c * sq, sq)],
                                start=True,
                                stop=True,
                            )
                        if i > 0:
                            self_i = i - 1
                            exp_prev = exs[self_i]
                            v_tile = v_sb[:, self_i, h * D:(h + 1) * D]
                            for c in range(sq_per2):
                                cw = bass.ds(c * sq, sq)
                                nc.tensor.matmul(
                                    av[:, cw],
                                    lhsT=v_tile,
                                    rhs=exp_prev[:, cw],
                                    start=(self_i == 0),
                                    stop=False,
                                )
                                nc.tensor.matmul(
                                    sm[:, cw],
                                    lhsT=ones_sb,
                                    rhs=exp_prev[:, cw],
                                    start=(self_i == 0),
                                    stop=False,
                                )
                        ex = expp.tile([P, sq2], BF16, tag="ex")
                        nc.scalar.activation(
                            ex, lg, mybir.ActivationFunctionType.Exp, scale=scale
                        )
                        exs[i] = ex
                        if debug_taps and h == 0 and j2 == 0 and i == 0:
                            exf = nrm.tile([P, sq2], F32, tag="dbgex")
                            nc.vector.tensor_copy(out=exf, in_=ex)
                            nc.sync.dma_start(out=taps["dbg_ex"], in_=exf)
                    # tail: last tile's accumulation
                    v_tile = v_sb[:, nst - 1, h * D:(h + 1) * D]
                    for c in range(sq_per2):
                        cw = bass.ds(c * sq, sq)
                        nc.tensor.matmul(
                            av[:, cw], lhsT=v_tile, rhs=exs[nst - 1][:, cw],
                            start=False, stop=True,
                        )
                        nc.tensor.matmul(
                            sm[:, cw], lhsT=ones_sb, rhs=exs[nst - 1][:, cw],
                            start=False, stop=True,
                        )
                    # free av/sm banks fast: plain copies off PSUM, then
                    # normalize from SBUF off the PE critical path.
                    avf = nrm.tile([P, sq2], F32, tag="avf")
                    nc.scalar.copy(avf, av)
                    recip = nrm.tile([1, sq2], F32, tag="recip")
                    nc.vector.reciprocal(recip, sm)
                    bcast = nrm.tile([P, sq2], F32, tag="bcast")
                    nc.gpsimd.partition_broadcast(bcast, recip)
                    if debug_taps and h == 0 and j2 == 0:
                        nc.sync.dma_start(out=taps["dbg_sm"], in_=recip)
                        nc.sync.dma_start(out=taps["dbg_av"], in_=avf)
                        nc.sync.dma_start(out=taps["dbg_bc"], in_=bcast)
                    nc.vector.tensor_mul(attn_sb[:, h, win2], avf, bcast)

        if debug_taps:
            nc.sync.dma_start(out=taps["dbg_q"], in_=q_sb)
            nc.sync.dma_start(out=taps["dbg_k"], in_=k_sb)
            nc.sync.dma_start(out=taps["dbg_v"], in_=v_sb)
            nc.sync.dma_start(out=taps["dbg_attn"], in_=attn_sb)

        # ================= Phase 3: out-projection =================
        outp = ctx.enter_context(tc.tile_pool(name="outp", bufs=3))
        nob = DIM // sq
        with tc.tile_pool(name="psO", bufs=2 * nob, space="PSUM") as psO:
            for st in range(nst):
                # h-inner-over-ob order: each attn stationary tile is loaded
                # once and reused for all nob matmuls.
                pos = [psO.tile([P, sq], F32, tag="po", name=f"po{ob}")
                        for ob in range(nob)]
                for h in range(HL_):
                    for ob in range(nob):
                        nc.tensor.matmul(
                            pos[ob],
                            lhsT=attn_sb[:, h, st * P:(st + 1) * P],
                            rhs=wo_sb[:, h, ob * sq:(ob + 1) * sq],
                            start=(h == 0),
                            stop=(h == HL_ - 1),
                        )
                for ob in range(nob):
                    ot = outp.tile([P, sq], F32, tag="ot")
                    if ob % 2 == 0:
                        nc.scalar.copy(ot, pos[ob])
                    else:
                        nc.vector.tensor_copy(out=ot, in_=pos[ob])
                    nc.sync.dma_start(
                        out=out_t[st][:, ob * sq:(ob + 1) * sq], in_=ot
                    )

    nc.compile()
    return nc


# ---------------- host side ----------------

def _rope_tables(S):
    inv_freq = 1.0 / (10000.0 ** (np.arange(0, D, 2, dtype=np.float32) / D))
    t = np.arange(S, dtype=np.float32)
    freqs = np.einsum("i,j->ij", t, inv_freq)      # [S, 64]
    cos_h = np.cos(freqs).T                        # [64, S]
    sin_h = np.sin(freqs).T
    cosf = np.concatenate([cos_h, cos_h], 0)       # [128, S]
    sinfs = np.concatenate([-sin_h, sin_h], 0)     # sign-folded
    return cosf, sinfs


def _shard_inputs(x, w_qkv, w_out):
    B, S, DIM = x.shape
    bf = ml_dtypes.bfloat16
    nkt = DIM // P
    cosf, sinfs = _rope_tables(S)
    cosf = cosf.astype(bf)
    sinfs = sinfs.astype(bf)
    in_maps = []
    for c in range(N_CORES):
        b, g = divmod(c, TP)
        h0 = HL * g
        # xt[p, kt, s] = x[b, s, kt*128+p]
        xt = np.ascontiguousarray(
            x[b].reshape(S, nkt, P).transpose(2, 1, 0)
        ).astype(bf)
        # wq[r, p, kt, m] = w_qkv[(h0+r)*128 + m, kt*128 + p]
        wq_s = w_qkv[h0 * D:(h0 + HL) * D]                  # [512, DIM]
        wk_s = w_qkv[HEADS * D + h0 * D:HEADS * D + (h0 + HL) * D]
        wv_s = w_qkv[2 * HEADS * D + h0 * D:2 * HEADS * D + (h0 + HL) * D]
        wq_t = np.ascontiguousarray(
            wq_s.reshape(HL, P, nkt, P).transpose(0, 3, 2, 1)
        ).astype(bf)
        wk_t = np.ascontiguousarray(
            wk_s.reshape(HL, P, nkt, P).transpose(0, 3, 2, 1)
        ).astype(bf)
        # wv[p, kt, vo] = wv_s[vo, kt*128+p]
        wv_t = np.ascontiguousarray(
            wv_s.reshape(HL * D, nkt, P).transpose(2, 1, 0)
        ).astype(bf)
        # wo[p, h, o] = w_out[o, 512g + h*128 + p]
        wo_s = w_out[:, h0 * D:(h0 + HL) * D]               # [DIM, 512]
        wo_t = np.ascontiguousarray(
            wo_s.reshape(DIM, HL, P).transpose(2, 1, 0)
        ).astype(bf)
        in_maps.append(
            {"xt": xt, "wq": wq_t, "wk": wk_t, "wv": wv_t, "wo": wo_t,
             "cosf": cosf, "sinfs": sinfs}
        )
    return in_maps


_NC_CACHE = {}


def _get_nc(S, DIM):
    key = (S, DIM)
    if key not in _NC_CACHE:
        _NC_CACHE[key] = build_kernel(S, DIM)
    return _NC_CACHE[key]


def kernel(x, w_qkv, w_out, trace=False):
    x = np.asarray(x)
    w_qkv = np.asarray(w_qkv)
    w_out = np.asarray(w_out)
    B, S, DIM = x.shape
    nc = _get_nc(S, DIM)
    in_maps = _shard_inputs(x, w_qkv, w_out)
    res = run_bass_kernel_spmd(nc, in_maps, core_ids=list(range(N_CORES)),
                               trace=trace)
    outs = [np.asarray(r["out"], dtype=np.float32) for r in res.results]
    full = np.stack(
        [sum(outs[b * TP:(b + 1) * TP][1:], outs[b * TP]) for b in range(DP)]
    ).astype(np.float32)
    if trace:
        kernel.last_results = res
    return full


# revision 24
# speedup vs baseline: 1.2844x; 1.2153x over previous
"""Multi-head attention (RoPE) Trainium2 kernel, 8 NeuronCores.

Sharding: data-parallel over batch (2) x tensor-parallel over heads (4
heads/core).  Core c handles batch c//4, heads 4*(c%4) .. 4*(c%4)+4.
Each core computes qkv projection for its heads, RoPE, full attention
over its heads, and the out-projection partial (w_out column shard).
The 4 partials per batch are summed on the host (TP all-reduce epilogue
done host-side; no device collective).

Device layouts (per core):
  xt    [128, DIM/128, S]    bf16   x[b].T tiled: xt[p, kt, s] = x[b, s, kt*128+p]
  wq/wk [r, 128, DIM/128, 128] bf16 stationary tiles for transposed proj
  wv    [128, DIM/128, HL*128] bf16 moving tiles for natural v proj
  wo    [128, HL, DIM]       bf16   wo[p, h, o] = w_out[o, 512g + h*128 + p]
  cosf/sinfs [128, S]        bf16   RoPE tables, transposed, halves duplicated,
                                    sin sign-folded (rows 0:64 negated)
  out   [S, DIM]             f32    partial output (natural layout)
"""

import math
import sys
from contextlib import ExitStack

import numpy as np

sys.path.insert(0, "/opt/trn_rl_repo")

import ml_dtypes  # noqa: E402

import concourse.bass as bass  # noqa: E402
import concourse.tile as tile  # noqa: E402
from concourse import bacc, mybir  # noqa: E402
from concourse.bass_utils import run_bass_kernel_spmd  # noqa: E402

P = 128          # partitions / head dim
HEADS = 16
D = 128
N_CORES = 8
DP = 2           # batch shards
TP = 4           # head-group shards
HL = HEADS // TP  # heads per core

BF16 = mybir.dt.bfloat16
F32 = mybir.dt.float32


def build_kernel(S, DIM, HL_=HL, sq=512, sq2=1024, num_devices=N_CORES,
                 debug_taps=False):
    """Build + compile the per-core Bass program (SPMD: same program all cores)."""
    nkt = DIM // P    # contraction tiles for projections
    nst = S // P      # sequence tiles of 128
    nsq = S // sq     # 512-wide free blocks
    sq2 = min(sq2, S)
    nsq2 = S // sq2   # attention s_q groups
    sq_per2 = sq2 // sq
    scale = 1.0 / math.sqrt(D)

    nc = bacc.Bacc("TRN2", debug=False, num_devices=num_devices)

    xt = nc.dram_tensor("xt", [P, nkt, S], BF16, kind="ExternalInput").ap()
    wq = nc.dram_tensor("wq", [HL_, P, nkt, P], BF16, kind="ExternalInput").ap()
    wk = nc.dram_tensor("wk", [HL_, P, nkt, P], BF16, kind="ExternalInput").ap()
    wv = nc.dram_tensor("wv", [P, nkt, HL_ * D], BF16, kind="ExternalInput").ap()
    wo = nc.dram_tensor("wo", [P, HL_, DIM], BF16, kind="ExternalInput").ap()
    cosf = nc.dram_tensor("cosf", [P, S], BF16, kind="ExternalInput").ap()
    sinfs = nc.dram_tensor("sinfs", [P, S], BF16, kind="ExternalInput").ap()
    out = nc.dram_tensor("out", [S, DIM], F32, kind="ExternalOutput").ap()
    out_t = out.rearrange("(st p) o -> st p o", p=P)
    taps = {}
    if debug_taps:
        for name, shape in (
            ("dbg_q", [P, HL_, S]), ("dbg_k", [P, HL_, S]),
            ("dbg_v", [P, S // P, HL_ * D]), ("dbg_attn", [P, HL_, S]),
        ):
            taps[name] = nc.dram_tensor(
                name, shape, BF16, kind="ExternalOutput").ap()
        for name, shape in (
            ("dbg_ex", [P, sq2]), ("dbg_sm", [1, sq2]),
            ("dbg_av", [P, sq2]), ("dbg_bc", [P, sq2]),
        ):
            taps[name] = nc.dram_tensor(
                name, shape, F32, kind="ExternalOutput").ap()

    with tile.TileContext(nc) as tc, ExitStack() as ctx:
        nc = tc.nc
        # ---- persistent SBUF ----
        big = ctx.enter_context(tc.tile_pool(name="big", bufs=1))
        q_sb = big.tile([P, HL_, S], BF16, tag="q")
        k_sb = big.tile([P, HL_, S], BF16, tag="k")
        v_sb = big.tile([P, nst, HL_ * D], BF16, tag="v")
        attn_sb = big.tile([P, HL_, S], BF16, tag="attn")
        cos_sb = big.tile([P, S], BF16, tag="cos")
        sin_sb = big.tile([P, S], BF16, tag="sin")
        ones_sb = big.tile([P, 1], BF16, tag="ones")
        wv_sb = big.tile([P, nkt, HL_ * D], BF16, tag="wv")
        wo_sb = big.tile([P, HL_, DIM], BF16, tag="wo")

        nc.vector.memset(ones_sb, 1.0)
        nc.sync.dma_start(out=cos_sb, in_=cosf)
        nc.sync.dma_start(out=sin_sb, in_=sinfs)

        xpool = ctx.enter_context(tc.tile_pool(name="xpool", bufs=2))
        wpool = ctx.enter_context(tc.tile_pool(name="wpool", bufs=3))
        rope = ctx.enter_context(tc.tile_pool(name="rope", bufs=2))

        # ================= Phase 1: qkv projection + RoPE =================
        # x streamed in sq-wide chunks; all 12 output row-tiles per chunk.
        spt = sq // P  # s-tiles of 128 per chunk
        with tc.tile_pool(name="psA", bufs=4, space="PSUM") as psA:
            for j in range(nsq):
                win = bass.ds(j * sq, sq)
                xc = xpool.tile([P, nkt, sq], BF16, tag="xc")
                for kt in range(nkt):
                    nc.sync.dma_start(out=xc[:, kt, :], in_=xt[:, kt, win])
                # q and k (transposed orientation [d, s])
                for which, wdram, dst in (("q", wq, q_sb), ("k", wk, k_sb)):
                    for h in range(HL_):
                        w_t = wpool.tile([P, nkt, P], BF16, tag="w")
                        nc.sync.dma_start(out=w_t, in_=wdram[h])
                        ps = psA.tile([P, sq], F32, tag="ps")
                        for kt in range(nkt):
                            nc.tensor.matmul(
                                ps,
                                lhsT=w_t[:, kt, :],
                                rhs=xc[:, kt, :],
                                start=(kt == 0),
                                stop=(kt == nkt - 1),
                            )
                        # RoPE: dst = p*cos + swap(p)*sin_signed
                        pb = rope.tile([P, sq], BF16, tag="pb")
                        nc.scalar.copy(pb, ps)
                        sw = rope.tile([P, sq], BF16, tag="sw")
                        nc.vector.tensor_copy(out=sw[0:64, :], in_=pb[64:128, :])
                        nc.vector.tensor_copy(out=sw[64:128, :], in_=pb[0:64, :])
                        t1 = rope.tile([P, sq], BF16, tag="t1")
                        nc.vector.tensor_mul(t1, pb, cos_sb[:, win])
                        t2 = rope.tile([P, sq], BF16, tag="t2")
                        nc.vector.tensor_mul(t2, sw, sin_sb[:, win])
                        nc.vector.tensor_add(dst[:, h, win], t1, t2)
                # v (natural orientation [s, d_local])
                if j == 0:
                    nc.sync.dma_start(out=wv_sb, in_=wv)
                for sl in range(spt):
                    st = j * spt + sl
                    ps = psA.tile([P, HL_ * D], F32, tag="ps")
                    for kt in range(nkt):
                        nc.tensor.matmul(
                            ps,
                            lhsT=xc[:, kt, sl * P:(sl + 1) * P],
                            rhs=wv_sb[:, kt, :],
                            start=(kt == 0),
                            stop=(kt == nkt - 1),
                        )
                    nc.scalar.copy(v_sb[:, st, :], ps)

        # ================= Phase 2: attention =================
        expp = ctx.enter_context(tc.tile_pool(name="expp", bufs=6))
        nrm = ctx.enter_context(tc.tile_pool(name="nrm", bufs=1))
        with (
            tc.tile_pool(name="psLG", bufs=2, space="PSUM") as psLG,
            tc.tile_pool(name="psAV", bufs=1, space="PSUM") as psAV,
            tc.tile_pool(name="psSM", bufs=1, space="PSUM") as psSM,
        ):
            PK = min(4, nst)  # ones-matmuls packed per column-tiled group
            for h in range(HL_):
                for j2 in range(nsq2):
                    win2 = bass.ds(j2 * sq2, sq2)
                    av = psAV.tile([P, sq2], F32, tag="av")
                    # sums live in 4 partition rows (0/32/64/96) per c-half:
                    # packed col-tiled ones-matmuls run concurrently on PE.
                    sm = psSM.tile([P, sq2], F32, tag="sm")
                    # software-pipelined: av for tile i-1 issues after
                    # logits for tile i, so PE never queues behind exp.
                    exs = [None] * nst
                    for i in range(nst):
                        lg = psLG.tile([P, sq2], F32, tag="lg")
                        k_tile = k_sb[:, h, i * P:(i + 1) * P]
                        for c in range(sq_per2):
                            nc.tensor.matmul(
                                lg[:, c * sq:(c + 1) * sq],
                                lhsT=k_tile,
                                rhs=q_sb[:, h, bass.ds(j2 * sq2 + c * sq, sq)],
                                start=True,
                                stop=True,
                            )
                        if i > 0:
                            pi = i - 1
                            exp_prev = exs[pi]
                            v_tile = v_sb[:, pi, h * D:(h + 1) * D]
                            for c in range(sq_per2):
                                nc.tensor.matmul(
                                    av[:, bass.ds(c * sq, sq)],
                                    lhsT=v_tile,
                                    rhs=exp_prev[:, bass.ds(c * sq, sq)],
                                    start=(pi == 0),
                                    stop=(pi == nst - 1),
                                )
                        if i > 0 and i % PK == 0:
                            for c in range(sq_per2):
                                cw = bass.ds(c * sq, sq)
                                for r in range(PK):
                                    ii = i - PK + r
                                    nc.tensor.matmul(
                                        sm[32 * r:32 * r + 1, cw],
                                        lhsT=ones_sb,
                                        rhs=exs[ii][:, cw],
                                        start=(ii < PK),
                                        stop=False,
                                        tile_position=(0, 32 * r),
                                    )
                        ex = expp.tile([P, sq2], BF16, tag="ex")
                        nc.scalar.activation(
                            ex, lg, mybir.ActivationFunctionType.Exp, scale=scale
                        )
                        exs[i] = ex
                        if debug_taps and h == 0 and j2 == 0 and i == 0:
                            exf = nrm.tile([P, sq2], F32, tag="dbgex")
                            nc.vector.tensor_copy(out=exf, in_=ex)
                            nc.sync.dma_start(out=taps["dbg_ex"], in_=exf)
                    # tail: last tile's av accumulation + final sums pack
                    v_tile = v_sb[:, nst - 1, h * D:(h + 1) * D]
                    for c in range(sq_per2):
                        cw = bass.ds(c * sq, sq)
                        nc.tensor.matmul(
                            av[:, cw], lhsT=v_tile, rhs=exs[nst - 1][:, cw],
                            start=False, stop=True,
                        )
                    for c in range(sq_per2):
                        cw = bass.ds(c * sq, sq)
                        for r in range(PK):
                            ii = nst - PK + r
                            nc.tensor.matmul(
                                sm[32 * r:32 * r + 1, cw],
                                lhsT=ones_sb,
                                rhs=exs[ii][:, cw],
                                start=(ii < PK),
                                stop=True,
                                tile_position=(0, 32 * r),
                            )
                    # free av/sm banks fast (cheap DVE ops), then normalize
                    # off the PE critical path.
                    avf = nrm.tile([P, sq2], F32, tag="avf")
                    nc.vector.tensor_copy(out=avf, in_=av)
                    # combine the PK partial-sum rows into SBUF (DVE can
                    # read at most one PSUM operand per instruction)
                    ssum = nrm.tile([1, sq2], F32, tag="ssum")
                    nc.vector.tensor_copy(out=ssum, in_=sm[0:1, :])
                    for r in range(1, PK):
                        nc.vector.tensor_add(
                            ssum, ssum, sm[32 * r:32 * r + 1, :]
                        )
                    recip = nrm.tile([1, sq2], F32, tag="recip")
                    nc.vector.reciprocal_approx_fast(out=recip, in_=ssum)
                    bcast = nrm.tile([P, sq2], F32, tag="bcast")
                    nc.gpsimd.partition_broadcast(bcast, recip)
                    if debug_taps and h == 0 and j2 == 0:
                        nc.sync.dma_start(out=taps["dbg_sm"], in_=recip)
                        nc.sync.dma_start(out=taps["dbg_av"], in_=avf)
                        nc.sync.dma_start(out=taps["dbg_bc"], in_=bcast)
                    nc.vector.tensor_mul(attn_sb[:, h, win2], avf, bcast)

        if debug_taps:
            nc.sync.dma_start(out=taps["dbg_q"], in_=q_sb)
            nc.sync.dma_start(out=taps["dbg_k"], in_=k_sb)
            nc.sync.dma_start(out=taps["dbg_v"], in_=v_sb)
            nc.sync.dma_start(out=taps["dbg_attn"], in_=attn_sb)

        # ================= Phase 3: out-projection =================
        outp = ctx.enter_context(tc.tile_pool(name="outp", bufs=3))
        nob = DIM // sq
        nc.sync.dma_start(out=wo_sb, in_=wo)
        with tc.tile_pool(name="psO", bufs=2 * nob, space="PSUM") as psO:
            for st in range(nst):
                # h-inner-over-ob order: each attn stationary tile is loaded
                # once and reused for all nob matmuls.
                pos = [psO.tile([P, sq], F32, tag="po", name=f"po{ob}")
                        for ob in range(nob)]
                for h in range(HL_):
                    for ob in range(nob):
                        nc.tensor.matmul(
                            pos[ob],
                            lhsT=attn_sb[:, h, st * P:(st + 1) * P],
                            rhs=wo_sb[:, h, ob * sq:(ob + 1) * sq],
                            start=(h == 0),
                            stop=(h == HL_ - 1),
                        )
                for ob in range(nob):
                    ot = outp.tile([P, sq], F32, tag="ot")
                    nc.vector.tensor_copy(out=ot, in_=pos[ob])
                    nc.sync.dma_start(
                        out=out_t[st][:, ob * sq:(ob + 1) * sq], in_=ot
                    )

    nc.compile()
    return nc


# ---------------- host side ----------------

def _rope_tables(S):
    inv_freq = 1.0 / (10000.0 ** (np.arange(0, D, 2, dtype=np.float32) / D))
    t = np.arange(S, dtype=np.float32)
    freqs = np.einsum("i,j->ij", t, inv_freq)      # [S, 64]
    cos_h = np.cos(freqs).T                        # [64, S]
    sin_h = np.sin(freqs).T
    cosf = np.concatenate([cos_h, cos_h], 0)       # [128, S]
    sinfs = np.concatenate([-sin_h, sin_h], 0)     # sign-folded
    return cosf, sinfs


def _shard_inputs(x, w_qkv, w_out):
    B, S, DIM = x.shape
    bf = ml_dtypes.bfloat16
    nkt = DIM // P
    cosf, sinfs = _rope_tables(S)
    cosf = cosf.astype(bf)
    sinfs = sinfs.astype(bf)
    in_maps = []
    for c in range(N_CORES):
        b, g = divmod(c, TP)
        h0 = HL * g
        # xt[p, kt, s] = x[b, s, kt*128+p]
        xt = np.ascontiguousarray(
            x[b].reshape(S, nkt, P).transpose(2, 1, 0)
        ).astype(bf)
        # wq[r, p, kt, m] = w_qkv[(h0+r)*128 + m, kt*128 + p]
        wq_s = w_qkv[h0 * D:(h0 + HL) * D]                  # [512, DIM]
        wk_s = w_qkv[HEADS * D + h0 * D:HEADS * D + (h0 + HL) * D]
        wv_s = w_qkv[2 * HEADS * D + h0 * D:2 * HEADS * D + (h0 + HL) * D]
        wq_t = np.ascontiguousarray(
            wq_s.reshape(HL, P, nkt, P).transpose(0, 3, 2, 1)
        ).astype(bf)
        wk_t = np.ascontiguousarray(
            wk_s.reshape(HL, P, nkt, P).transpose(0, 3, 2, 1)
        ).astype(bf)
        # wv[p, kt, vo] = wv_s[vo, kt*128+p]
        wv_t = np.ascontiguousarray(
            wv_s.reshape(HL * D, nkt, P).transpose(2, 1, 0)
        ).astype(bf)
        # wo[p, h, o] = w_out[o, 512g + h*128 + p]
        wo_s = w_out[:, h0 * D:(h0 + HL) * D]               # [DIM, 512]
        wo_t = np.ascontiguousarray(
            wo_s.reshape(DIM, HL, P).transpose(2, 1, 0)
        ).astype(bf)
        in_maps.append(
            {"xt": xt, "wq": wq_t, "wk": wk_t, "wv": wv_t, "wo": wo_t,
             "cosf": cosf, "sinfs": sinfs}
        )
    return in_maps


_NC_CACHE = {}


def _get_nc(S, DIM):
    key = (S, DIM)
    if key not in _NC_CACHE:
        _NC_CACHE[key] = build_kernel(S, DIM)
    return _NC_CACHE[key]


def kernel(x, w_qkv, w_out, trace=False):
    x = np.asarray(x)
    w_qkv = np.asarray(w_qkv)
    w_out = np.asarray(w_out)
    B, S, DIM = x.shape
    nc = _get_nc(S, DIM)
    in_maps = _shard_inputs(x, w_qkv, w_out)
    res = run_bass_kernel_spmd(nc, in_maps, core_ids=list(range(N_CORES)),
                               trace=trace)
    outs = [np.asarray(r["out"], dtype=np.float32) for r in res.results]
    full = np.stack(
        [sum(outs[b * TP:(b + 1) * TP][1:], outs[b * TP]) for b in range(DP)]
    ).astype(np.float32)
    if trace:
        kernel.last_results = res
    return full


# revision 25
# speedup vs baseline: 1.2865x; 1.0017x over previous
"""Multi-head attention (RoPE) Trainium2 kernel, 8 NeuronCores.

Sharding: data-parallel over batch (2) x tensor-parallel over heads (4
heads/core).  Core c handles batch c//4, heads 4*(c%4) .. 4*(c%4)+4.
Each core computes qkv projection for its heads, RoPE, full attention
over its heads, and the out-projection partial (w_out column shard).
The 4 partials per batch are summed on the host (TP all-reduce epilogue
done host-side; no device collective).

Device layouts (per core):
  xt    [128, DIM/128, S]    bf16   x[b].T tiled: xt[p, kt, s] = x[b, s, kt*128+p]
  wq/wk [r, 128, DIM/128, 128] bf16 stationary tiles for transposed proj
  wv    [128, DIM/128, HL*128] bf16 moving tiles for natural v proj
  wo    [128, HL, DIM]       bf16   wo[p, h, o] = w_out[o, 512g + h*128 + p]
  cosf/sinfs [128, S]        bf16   RoPE tables, transposed, halves duplicated,
                                    sin sign-folded (rows 0:64 negated)
  out   [S, DIM]             f32    partial output (natural layout)
"""

import math
import sys
from contextlib import ExitStack

import numpy as np

sys.path.insert(0, "/opt/trn_rl_repo")

import ml_dtypes  # noqa: E402

import concourse.bass as bass  # noqa: E402
import concourse.tile as tile  # noqa: E402
from concourse import bacc, mybir  # noqa: E402
from concourse.bass_utils import run_bass_kernel_spmd  # noqa: E402

P = 128          # partitions / head dim
HEADS = 16
D = 128
N_CORES = 8
DP = 2           # batch shards
TP = 4           # head-group shards
HL = HEADS // TP  # heads per core

BF16 = mybir.dt.bfloat16
F32 = mybir.dt.float32


def build_kernel(S, DIM, HL_=HL, sq=512, sq2=1024, num_devices=N_CORES,
                 debug_taps=False):
    """Build + compile the per-core Bass program (SPMD: same program all cores)."""
    nkt = DIM // P    # contraction tiles for projections
    nst = S // P      # sequence tiles of 128
    nsq = S // sq     # 512-wide free blocks
    sq2 = min(sq2, S)
    nsq2 = S // sq2   # attention s_q groups
    sq_per2 = sq2 // sq
    scale = 1.0 / math.sqrt(D)

    nc = bacc.Bacc("TRN2", debug=False, num_devices=num_devices)

    xt = nc.dram_tensor("xt", [P, nkt, S], BF16, kind="ExternalInput").ap()
    wq = nc.dram_tensor("wq", [HL_, P, nkt, P], BF16, kind="ExternalInput").ap()
    wk = nc.dram_tensor("wk", [HL_, P, nkt, P], BF16, kind="ExternalInput").ap()
    wv = nc.dram_tensor("wv", [P, nkt, HL_ * D], BF16, kind="ExternalInput").ap()
    wo = nc.dram_tensor("wo", [P, HL_, DIM], BF16, kind="ExternalInput").ap()
    cosf = nc.dram_tensor("cosf", [P, S], BF16, kind="ExternalInput").ap()
    sinfs = nc.dram_tensor("sinfs", [P, S], BF16, kind="ExternalInput").ap()
    out = nc.dram_tensor("out", [S, DIM], F32, kind="ExternalOutput").ap()
    out_t = out.rearrange("(st p) o -> st p o", p=P)
    taps = {}
    if debug_taps:
        for name, shape in (
            ("dbg_q", [P, HL_, S]), ("dbg_k", [P, HL_, S]),
            ("dbg_v", [P, S // P, HL_ * D]), ("dbg_attn", [P, HL_, S]),
        ):
            taps[name] = nc.dram_tensor(
                name, shape, BF16, kind="ExternalOutput").ap()
        for name, shape in (
            ("dbg_ex", [P, sq2]), ("dbg_sm", [1, sq2]),
            ("dbg_av", [P, sq2]), ("dbg_bc", [P, sq2]),
        ):
            taps[name] = nc.dram_tensor(
                name, shape, F32, kind="ExternalOutput").ap()

    with tile.TileContext(nc) as tc, ExitStack() as ctx:
        nc = tc.nc
        # ---- persistent SBUF ----
        big = ctx.enter_context(tc.tile_pool(name="big", bufs=1))
        q_sb = big.tile([P, HL_, S], BF16, tag="q")
        k_sb = big.tile([P, HL_, S], BF16, tag="k")
        v_sb = big.tile([P, nst, HL_ * D], BF16, tag="v")
        attn_sb = big.tile([P, HL_, S], BF16, tag="attn")
        cos_sb = big.tile([P, S], BF16, tag="cos")
        sin_sb = big.tile([P, S], BF16, tag="sin")
        ones_sb = big.tile([P, 1], BF16, tag="ones")
        wv_sb = big.tile([P, nkt, HL_ * D], BF16, tag="wv")
        wo_sb = big.tile([P, HL_, DIM], BF16, tag="wo")

        nc.vector.memset(ones_sb, 1.0)

        xpool = ctx.enter_context(tc.tile_pool(name="xpool", bufs=2))
        wpool = ctx.enter_context(tc.tile_pool(name="wpool", bufs=3))
        rope = ctx.enter_context(tc.tile_pool(name="rope", bufs=2))

        # ================= Phase 1: qkv projection + RoPE =================
        # x streamed in sq-wide chunks; all 12 output row-tiles per chunk.
        spt = sq // P  # s-tiles of 128 per chunk
        with tc.tile_pool(name="psA", bufs=4, space="PSUM") as psA:
            for j in range(nsq):
                win = bass.ds(j * sq, sq)
                xc = xpool.tile([P, nkt, sq], BF16, tag="xc")
                for kt in range(nkt):
                    nc.sync.dma_start(out=xc[:, kt, :], in_=xt[:, kt, win])
                if j == 0:
                    nc.sync.dma_start(out=cos_sb, in_=cosf)
                    nc.sync.dma_start(out=sin_sb, in_=sinfs)
                # q and k (transposed orientation [d, s])
                for which, wdram, dst in (("q", wq, q_sb), ("k", wk, k_sb)):
                    for h in range(HL_):
                        w_t = wpool.tile([P, nkt, P], BF16, tag="w")
                        nc.sync.dma_start(out=w_t, in_=wdram[h])
                        ps = psA.tile([P, sq], F32, tag="ps")
                        for kt in range(nkt):
                            nc.tensor.matmul(
                                ps,
                                lhsT=w_t[:, kt, :],
                                rhs=xc[:, kt, :],
                                start=(kt == 0),
                                stop=(kt == nkt - 1),
                            )
                        # RoPE: dst = p*cos + swap(p)*sin_signed
                        pb = rope.tile([P, sq], BF16, tag="pb")
                        nc.scalar.copy(pb, ps)
                        sw = rope.tile([P, sq], BF16, tag="sw")
                        nc.vector.tensor_copy(out=sw[0:64, :], in_=pb[64:128, :])
                        nc.vector.tensor_copy(out=sw[64:128, :], in_=pb[0:64, :])
                        t1 = rope.tile([P, sq], BF16, tag="t1")
                        nc.vector.tensor_mul(t1, pb, cos_sb[:, win])
                        t2 = rope.tile([P, sq], BF16, tag="t2")
                        nc.vector.tensor_mul(t2, sw, sin_sb[:, win])
                        nc.vector.tensor_add(dst[:, h, win], t1, t2)
                # v (natural orientation [s, d_local])
                if j == 0:
                    nc.sync.dma_start(out=wv_sb, in_=wv)
                for sl in range(spt):
                    st = j * spt + sl
                    ps = psA.tile([P, HL_ * D], F32, tag="ps")
                    for kt in range(nkt):
                        nc.tensor.matmul(
                            ps,
                            lhsT=xc[:, kt, sl * P:(sl + 1) * P],
                            rhs=wv_sb[:, kt, :],
                            start=(kt == 0),
                            stop=(kt == nkt - 1),
                        )
                    nc.scalar.copy(v_sb[:, st, :], ps)

        # ================= Phase 2: attention =================
        expp = ctx.enter_context(tc.tile_pool(name="expp", bufs=4))
        nrm = ctx.enter_context(tc.tile_pool(name="nrm", bufs=1))
        with (
            tc.tile_pool(name="psLG", bufs=2, space="PSUM") as psLG,
            tc.tile_pool(name="psAV", bufs=1, space="PSUM") as psAV,
            tc.tile_pool(name="psSM", bufs=1, space="PSUM") as psSM,
        ):
            PK = min(2, nst)  # ones-matmuls packed per column-tiled group
            for h in range(HL_):
                for j2 in range(nsq2):
                    win2 = bass.ds(j2 * sq2, sq2)
                    av = psAV.tile([P, sq2], F32, tag="av")
                    # sums live in 4 partition rows (0/32/64/96) per c-half:
                    # packed col-tiled ones-matmuls run concurrently on PE.
                    sm = psSM.tile([P, sq2], F32, tag="sm")
                    # software-pipelined: av for tile i-1 issues after
                    # logits for tile i, so PE never queues behind exp.
                    exs = [None] * nst
                    for i in range(nst):
                        lg = psLG.tile([P, sq2], F32, tag="lg")
                        k_tile = k_sb[:, h, i * P:(i + 1) * P]
                        for c in range(sq_per2):
                            nc.tensor.matmul(
                                lg[:, c * sq:(c + 1) * sq],
                                lhsT=k_tile,
                                rhs=q_sb[:, h, bass.ds(j2 * sq2 + c * sq, sq)],
                                start=True,
                                stop=True,
                            )
                        if i > 0:
                            pi = i - 1
                            exp_prev = exs[pi]
                            v_tile = v_sb[:, pi, h * D:(h + 1) * D]
                            for c in range(sq_per2):
                                nc.tensor.matmul(
                                    av[:, bass.ds(c * sq, sq)],
                                    lhsT=v_tile,
                                    rhs=exp_prev[:, bass.ds(c * sq, sq)],
                                    start=(pi == 0),
                                    stop=(pi == nst - 1),
                                )
                        if i > 0 and i % PK == 0:
                            for c in range(sq_per2):
                                cw = bass.ds(c * sq, sq)
                                for r in range(PK):
                                    ii = i - PK + r
                                    nc.tensor.matmul(
                                        sm[32 * r:32 * r + 1, cw],
                                        lhsT=ones_sb,
                                        rhs=exs[ii][:, cw],
                                        start=(ii < PK),
                                        stop=False,
                                        tile_position=(0, 32 * r),
                                    )
                        ex = expp.tile([P, sq2], BF16, tag="ex")
                        nc.scalar.activation(
                            ex, lg, mybir.ActivationFunctionType.Exp, scale=scale
                        )
                        exs[i] = ex
                        if debug_taps and h == 0 and j2 == 0 and i == 0:
                            exf = nrm.tile([P, sq2], F32, tag="dbgex")
                            nc.vector.tensor_copy(out=exf, in_=ex)
                            nc.sync.dma_start(out=taps["dbg_ex"], in_=exf)
                    # tail: last tile's av accumulation + final sums pack
                    v_tile = v_sb[:, nst - 1, h * D:(h + 1) * D]
                    for c in range(sq_per2):
                        cw = bass.ds(c * sq, sq)
                        nc.tensor.matmul(
                            av[:, cw], lhsT=v_tile, rhs=exs[nst - 1][:, cw],
                            start=False, stop=True,
                        )
                    for c in range(sq_per2):
                        cw = bass.ds(c * sq, sq)
                        for r in range(PK):
                            ii = nst - PK + r
                            nc.tensor.matmul(
                                sm[32 * r:32 * r + 1, cw],
                                lhsT=ones_sb,
                                rhs=exs[ii][:, cw],
                                start=(ii < PK),
                                stop=True,
                                tile_position=(0, 32 * r),
                            )
                    # free av/sm banks fast (cheap DVE ops), then normalize
                    # off the PE critical path.
                    avf = nrm.tile([P, sq2], F32, tag="avf")
                    nc.vector.tensor_copy(out=avf, in_=av)
                    # combine the PK partial-sum rows into SBUF (DVE can
                    # read at most one PSUM operand per instruction)
                    ssum = nrm.tile([1, sq2], F32, tag="ssum")
                    nc.vector.tensor_copy(out=ssum, in_=sm[0:1, :])
                    for r in range(1, PK):
                        nc.vector.tensor_add(
                            ssum, ssum, sm[32 * r:32 * r + 1, :]
                        )
                    recip = nrm.tile([1, sq2], F32, tag="recip")
                    nc.vector.reciprocal_approx_fast(out=recip, in_=ssum)
                    bcast = nrm.tile([P, sq2], F32, tag="bcast")
                    nc.gpsimd.partition_broadcast(bcast, recip)
                    if debug_taps and h == 0 and j2 == 0:
                        nc.sync.dma_start(out=taps["dbg_sm"], in_=recip)
                        nc.sync.dma_start(out=taps["dbg_av"], in_=avf)
                        nc.sync.dma_start(out=taps["dbg_bc"], in_=bcast)
                    nc.vector.tensor_mul(attn_sb[:, h, win2], avf, bcast)

        if debug_taps:
            nc.sync.dma_start(out=taps["dbg_q"], in_=q_sb)
            nc.sync.dma_start(out=taps["dbg_k"], in_=k_sb)
            nc.sync.dma_start(out=taps["dbg_v"], in_=v_sb)
            nc.sync.dma_start(out=taps["dbg_attn"], in_=attn_sb)

        # ================= Phase 3: out-projection =================
        outp = ctx.enter_context(tc.tile_pool(name="outp", bufs=3))
        nob = DIM // sq
        nc.sync.dma_start(out=wo_sb, in_=wo)
        with tc.tile_pool(name="psO", bufs=6, space="PSUM") as psO:
            for st in range(nst):
                # h-inner-over-ob order: each attn stationary tile is loaded
                # once and reused for all nob matmuls.
                pos = [psO.tile([P, sq], F32, tag="po", name=f"po{ob}")
                        for ob in range(nob)]
                for h in range(HL_):
                    for ob in range(nob):
                        nc.tensor.matmul(
                            pos[ob],
                            lhsT=attn_sb[:, h, st * P:(st + 1) * P],
                            rhs=wo_sb[:, h, ob * sq:(ob + 1) * sq],
                            start=(h == 0),
                            stop=(h == HL_ - 1),
                        )
                for ob in range(nob):
                    ot = outp.tile([P, sq], F32, tag="ot")
                    nc.vector.tensor_copy(out=ot, in_=pos[ob])
                    nc.sync.dma_start(
                        out=out_t[st][:, ob * sq:(ob + 1) * sq], in_=ot
                    )

    nc.compile()
    return nc


# ---------------- host side ----------------

def _rope_tables(S):
    inv_freq = 1.0 / (10000.0 ** (np.arange(0, D, 2, dtype=np.float32) / D))
    t = np.arange(S, dtype=np.float32)
    freqs = np.einsum("i,j->ij", t, inv_freq)      # [S, 64]
    cos_h = np.cos(freqs).T                        # [64, S]
    sin_h = np.sin(freqs).T
    cosf = np.concatenate([cos_h, cos_h], 0)       # [128, S]
    sinfs = np.concatenate([-sin_h, sin_h], 0)     # sign-folded
    return cosf, sinfs


def _shard_inputs(x, w_qkv, w_out):
    B, S, DIM = x.shape
    bf = ml_dtypes.bfloat16
    nkt = DIM // P
    cosf, sinfs = _rope_tables(S)
    cosf = cosf.astype(bf)
    sinfs = sinfs.astype(bf)
    in_maps = []
    for c in range(N_CORES):
        b, g = divmod(c, TP)
        h0 = HL * g
        # xt[p, kt, s] = x[b, s, kt*128+p]
        xt = np.ascontiguousarray(
            x[b].reshape(S, nkt, P).transpose(2, 1, 0)
        ).astype(bf)
        # wq[r, p, kt, m] = w_qkv[(h0+r)*128 + m, kt*128 + p]
        wq_s = w_qkv[h0 * D:(h0 + HL) * D]                  # [512, DIM]
        wk_s = w_qkv[HEADS * D + h0 * D:HEADS * D + (h0 + HL) * D]
        wv_s = w_qkv[2 * HEADS * D + h0 * D:2 * HEADS * D + (h0 + HL) * D]
        wq_t = np.ascontiguousarray(
            wq_s.reshape(HL, P, nkt, P).transpose(0, 3, 2, 1)
        ).astype(bf)
        wk_t = np.ascontiguousarray(
            wk_s.reshape(HL, P, nkt, P).transpose(0, 3, 2, 1)
        ).astype(bf)
        # wv[p, kt, vo] = wv_s[vo, kt*128+p]
        wv_t = np.ascontiguousarray(
            wv_s.reshape(HL * D, nkt, P).transpose(2, 1, 0)
        ).astype(bf)
        # wo[p, h, o] = w_out[o, 512g + h*128 + p]
        wo_s = w_out[:, h0 * D:(h0 + HL) * D]               # [DIM, 512]
        wo_t = np.ascontiguousarray(
            wo_s.reshape(DIM, HL, P).transpose(2, 1, 0)
        ).astype(bf)
        in_maps.append(
            {"xt": xt, "wq": wq_t, "wk": wk_t, "wv": wv_t, "wo": wo_t,
             "cosf": cosf, "sinfs": sinfs}
        )
    return in_maps


_NC_CACHE = {}


def _get_nc(S, DIM):
    key = (S, DIM)
    if key not in _NC_CACHE:
        _NC_CACHE[key] = build_kernel(S, DIM)
    return _NC_CACHE[key]


def kernel(x, w_qkv, w_out, trace=False):
    x = np.asarray(x)
    w_qkv = np.asarray(w_qkv)
    w_out = np.asarray(w_out)
    B, S, DIM = x.shape
    nc = _get_nc(S, DIM)
    in_maps = _shard_inputs(x, w_qkv, w_out)
    res = run_bass_kernel_spmd(nc, in_maps, core_ids=list(range(N_CORES)),
                               trace=trace)
    outs = [np.asarray(r["out"], dtype=np.float32) for r in res.results]
    full = np.stack(
        [sum(outs[b * TP:(b + 1) * TP][1:], outs[b * TP]) for b in range(DP)]
    ).astype(np.float32)
    if trace:
        kernel.last_results = res
    return full


# revision 28
# speedup vs baseline: 1.3659x; 1.0617x over previous
"""Multi-head attention (RoPE) Trainium2 kernel, 8 NeuronCores.

Sharding: data-parallel over batch (2) x tensor-parallel over heads (4
heads/core).  Core c handles batch c//4, heads 4*(c%4) .. 4*(c%4)+4.
Each core computes qkv projection for its heads, RoPE, full attention
over its heads, and the out-projection partial (w_out column shard).
The 4 partials per batch are summed on the host (TP all-reduce epilogue
done host-side; no device collective).

Device layouts (per core):
  xt    [128, DIM/128, S]    bf16   x[b].T tiled: xt[p, kt, s] = x[b, s, kt*128+p]
  wq/wk [r, 128, DIM/128, 128] bf16 stationary tiles for transposed proj
  wv    [128, DIM/128, HL*128] bf16 moving tiles for natural v proj
  wo    [128, HL, DIM]       bf16   wo[p, h, o] = w_out[o, 512g + h*128 + p]
  cosf/sinfs [128, S]        bf16   RoPE tables, transposed, halves duplicated,
                                    sin sign-folded (rows 0:64 negated)
  out   [S, DIM]             f32    partial output (natural layout)
"""

import math
import sys
from contextlib import ExitStack

import numpy as np

sys.path.insert(0, "/opt/trn_rl_repo")

import ml_dtypes  # noqa: E402

import concourse.bass as bass  # noqa: E402
import concourse.tile as tile  # noqa: E402
from concourse import bacc, mybir  # noqa: E402
from concourse.bass_utils import run_bass_kernel_spmd  # noqa: E402

P = 128          # partitions / head dim
HEADS = 16
D = 128
N_CORES = 8
DP = 2           # batch shards
TP = 4           # head-group shards
HL = HEADS // TP  # heads per core

BF16 = mybir.dt.bfloat16
F32 = mybir.dt.float32


def build_kernel(S, DIM, HL_=HL, sq=512, sq2=1024, num_devices=N_CORES,
                 debug_taps=False):
    """Build + compile the per-core Bass program (SPMD: same program all cores)."""
    nkt = DIM // P    # contraction tiles for projections
    nst = S // P      # sequence tiles of 128
    nsq = S // sq     # 512-wide free blocks
    sq2 = min(sq2, S)
    nsq2 = S // sq2   # attention s_q groups
    sq_per2 = sq2 // sq
    scale = 1.0 / math.sqrt(D)

    nc = bacc.Bacc("TRN2", debug=False, num_devices=num_devices)

    xt = nc.dram_tensor("xt", [P, nkt, S], BF16, kind="ExternalInput").ap()
    wq = nc.dram_tensor("wq", [HL_, P, nkt, P], BF16, kind="ExternalInput").ap()
    wk = nc.dram_tensor("wk", [HL_, P, nkt, P], BF16, kind="ExternalInput").ap()
    wv = nc.dram_tensor("wv", [P, nkt, HL_ * D], BF16, kind="ExternalInput").ap()
    wo = nc.dram_tensor("wo", [P, HL_, DIM], BF16, kind="ExternalInput").ap()
    cosf = nc.dram_tensor("cosf", [P, S], BF16, kind="ExternalInput").ap()
    sinfs = nc.dram_tensor("sinfs", [P, S], BF16, kind="ExternalInput").ap()
    out = nc.dram_tensor("out", [S, DIM], F32, kind="ExternalOutput").ap()
    out_t = out.rearrange("(st p) o -> st p o", p=P)
    taps = {}
    if debug_taps:
        for name, shape in (
            ("dbg_q", [P, HL_, S]), ("dbg_k", [P, HL_, S]),
            ("dbg_v", [P, S // P, HL_ * D]), ("dbg_attn", [P, HL_, S]),
        ):
            taps[name] = nc.dram_tensor(
                name, shape, BF16, kind="ExternalOutput").ap()
        for name, shape in (
            ("dbg_ex", [P, sq2]), ("dbg_sm", [1, sq2]),
            ("dbg_av", [P, sq2]), ("dbg_bc", [P, sq2]),
        ):
            taps[name] = nc.dram_tensor(
                name, shape, F32, kind="ExternalOutput").ap()

    with tile.TileContext(nc) as tc, ExitStack() as ctx:
        nc = tc.nc
        # ---- persistent SBUF ----
        big = ctx.enter_context(tc.tile_pool(name="big", bufs=1))
        q_sb = big.tile([P, HL_, S], BF16, tag="q")
        k_sb = big.tile([P, HL_, S], BF16, tag="k")
        v_sb = big.tile([P, nst, HL_ * D], BF16, tag="v")
        attn_sb = big.tile([P, HL_, S], BF16, tag="attn")
        cos_sb = big.tile([P, S], BF16, tag="cos")
        sin_sb = big.tile([P, S], BF16, tag="sin")
        ones_sb = big.tile([P, 1], BF16, tag="ones")
        wv_sb = big.tile([P, nkt, HL_ * D], BF16, tag="wv")
        wo_sb = big.tile([P, HL_, DIM], BF16, tag="wo")

        nc.vector.memset(ones_sb, 1.0)

        xpool = ctx.enter_context(tc.tile_pool(name="xpool", bufs=2))
        wpool = ctx.enter_context(tc.tile_pool(name="wpool", bufs=3))
        rope = ctx.enter_context(tc.tile_pool(name="rope", bufs=2))

        # ================= Phase 1: qkv projection + RoPE =================
        # x streamed in sq-wide chunks; all 12 output row-tiles per chunk.
        spt = sq // P  # s-tiles of 128 per chunk
        with tc.tile_pool(name="psA", bufs=4, space="PSUM") as psA:
            for j in range(nsq):
                win = bass.ds(j * sq, sq)
                xc = xpool.tile([P, nkt, sq], BF16, tag="xc")
                for kt in range(nkt):
                    nc.sync.dma_start(out=xc[:, kt, :], in_=xt[:, kt, win])
                if j == 0:
                    nc.sync.dma_start(out=cos_sb, in_=cosf)
                    nc.sync.dma_start(out=sin_sb, in_=sinfs)
                # q and k (transposed orientation [d, s])
                for which, wdram, dst in (("q", wq, q_sb), ("k", wk, k_sb)):
                    for h in range(HL_):
                        w_t = wpool.tile([P, nkt, P], BF16, tag="w")
                        nc.sync.dma_start(out=w_t, in_=wdram[h])
                        ps = psA.tile([P, sq], F32, tag="ps")
                        for kt in range(nkt):
                            nc.tensor.matmul(
                                ps,
                                lhsT=w_t[:, kt, :],
                                rhs=xc[:, kt, :],
                                start=(kt == 0),
                                stop=(kt == nkt - 1),
                            )
                        # RoPE: dst = p*cos + swap(p)*sin_signed
                        pb = rope.tile([P, sq], BF16, tag="pb")
                        nc.scalar.copy(pb, ps)
                        sw = rope.tile([P, sq], BF16, tag="sw")
                        nc.vector.tensor_copy(out=sw[0:64, :], in_=pb[64:128, :])
                        nc.vector.tensor_copy(out=sw[64:128, :], in_=pb[0:64, :])
                        t1 = rope.tile([P, sq], BF16, tag="t1")
                        nc.vector.tensor_mul(t1, pb, cos_sb[:, win])
                        t2 = rope.tile([P, sq], BF16, tag="t2")
                        nc.vector.tensor_mul(t2, sw, sin_sb[:, win])
                        nc.vector.tensor_add(dst[:, h, win], t1, t2)
                # v (natural orientation [s, d_local])
                if j == 0:
                    nc.sync.dma_start(out=wv_sb, in_=wv)
                for sl in range(spt):
                    st = j * spt + sl
                    ps = psA.tile([P, HL_ * D], F32, tag="ps")
                    for kt in range(nkt):
                        nc.tensor.matmul(
                            ps,
                            lhsT=xc[:, kt, sl * P:(sl + 1) * P],
                            rhs=wv_sb[:, kt, :],
                            start=(kt == 0),
                            stop=(kt == nkt - 1),
                        )
                    nc.scalar.copy(v_sb[:, st, :], ps)

        # ================= Phase 2: attention =================
        expp = ctx.enter_context(tc.tile_pool(name="expp", bufs=6))
        nrm = ctx.enter_context(tc.tile_pool(name="nrm", bufs=1))
        with (
            tc.tile_pool(name="psLG", bufs=2, space="PSUM") as psLG,
            tc.tile_pool(name="psAV", bufs=1, space="PSUM") as psAV,
            tc.tile_pool(name="psSM", bufs=1, space="PSUM") as psSM,
        ):
            PK = min(4, nst)  # ones-matmuls packed per column-tiled group
            for h in range(HL_):
                for j2 in range(nsq2):
                    win2 = bass.ds(j2 * sq2, sq2)
                    av = psAV.tile([P, sq2], F32, tag="av")
                    # sums live in 4 partition rows (0/32/64/96) per c-half:
                    # packed col-tiled ones-matmuls run concurrently on PE.
                    sm = psSM.tile([P, sq2], F32, tag="sm")
                    # software-pipelined: av for tile i-1 issues after
                    # logits for tile i, so PE never queues behind exp.
                    exs = [None] * nst
                    for i in range(nst):
                        lg = psLG.tile([P, sq2], F32, tag="lg")
                        k_tile = k_sb[:, h, i * P:(i + 1) * P]
                        for c in range(sq_per2):
                            nc.tensor.matmul(
                                lg[:, c * sq:(c + 1) * sq],
                                lhsT=k_tile,
                                rhs=q_sb[:, h, bass.ds(j2 * sq2 + c * sq, sq)],
                                start=True,
                                stop=True,
                            )
                        if i > 1:
                            pi = i - 2
                            exp_prev = exs[pi]
                            v_tile = v_sb[:, pi, h * D:(h + 1) * D]
                            for c in range(sq_per2):
                                nc.tensor.matmul(
                                    av[:, bass.ds(c * sq, sq)],
                                    lhsT=v_tile,
                                    rhs=exp_prev[:, bass.ds(c * sq, sq)],
                                    start=(pi == 0),
                                    stop=(pi == nst - 1),
                                )
                        if i > 0 and i % PK == 0:
                            for c in range(sq_per2):
                                cw = bass.ds(c * sq, sq)
                                for r in range(PK):
                                    ii = i - PK + r
                                    nc.tensor.matmul(
                                        sm[32 * r:32 * r + 1, cw],
                                        lhsT=ones_sb,
                                        rhs=exs[ii][:, cw],
                                        start=(ii < PK),
                                        stop=False,
                                        tile_position=(0, 32 * r),
                                    )
                        ex = expp.tile([P, sq2], BF16, tag="ex")
                        nc.scalar.activation(
                            ex, lg, mybir.ActivationFunctionType.Exp, scale=scale
                        )
                        exs[i] = ex
                        if debug_taps and h == 0 and j2 == 0 and i == 0:
                            exf = nrm.tile([P, sq2], F32, tag="dbgex")
                            nc.vector.tensor_copy(out=exf, in_=ex)
                            nc.sync.dma_start(out=taps["dbg_ex"], in_=exf)
                    # tail: last two tiles' av accumulation + final sums pack
                    for pi in (nst - 2, nst - 1):
                        v_tile = v_sb[:, pi, h * D:(h + 1) * D]
                        for c in range(sq_per2):
                            cw = bass.ds(c * sq, sq)
                            nc.tensor.matmul(
                                av[:, cw], lhsT=v_tile, rhs=exs[pi][:, cw],
                                start=(pi == 0), stop=(pi == nst - 1),
                            )
                    for c in range(sq_per2):
                        cw = bass.ds(c * sq, sq)
                        for r in range(PK):
                            ii = nst - PK + r
                            nc.tensor.matmul(
                                sm[32 * r:32 * r + 1, cw],
                                lhsT=ones_sb,
                                rhs=exs[ii][:, cw],
                                start=(ii < PK),
                                stop=True,
                                tile_position=(0, 32 * r),
                            )
                    # free av/sm banks fast (cheap DVE ops), then normalize
                    # off the PE critical path.
                    avf = nrm.tile([P, sq2], F32, tag="avf")
                    nc.vector.tensor_copy(out=avf, in_=av)
                    # combine the PK partial-sum rows into SBUF (DVE can
                    # read at most one PSUM operand per instruction)
                    ssum = nrm.tile([1, sq2], F32, tag="ssum")
                    nc.vector.tensor_copy(out=ssum, in_=sm[0:1, :])
                    for r in range(1, PK):
                        nc.vector.tensor_add(
                            ssum, ssum, sm[32 * r:32 * r + 1, :]
                        )
                    recip = nrm.tile([1, sq2], F32, tag="recip")
                    nc.vector.reciprocal_approx_fast(out=recip, in_=ssum)
                    bcast = nrm.tile([P, sq2], F32, tag="bcast")
                    nc.gpsimd.partition_broadcast(bcast, recip)
                    if debug_taps and h == 0 and j2 == 0:
                        nc.sync.dma_start(out=taps["dbg_sm"], in_=recip)
                        nc.sync.dma_start(out=taps["dbg_av"], in_=avf)
                        nc.sync.dma_start(out=taps["dbg_bc"], in_=bcast)
                    nc.vector.tensor_mul(attn_sb[:, h, win2], avf, bcast)

        if debug_taps:
            nc.sync.dma_start(out=taps["dbg_q"], in_=q_sb)
            nc.sync.dma_start(out=taps["dbg_k"], in_=k_sb)
            nc.sync.dma_start(out=taps["dbg_v"], in_=v_sb)
            nc.sync.dma_start(out=taps["dbg_attn"], in_=attn_sb)

        # ================= Phase 3: out-projection =================
        outp = ctx.enter_context(tc.tile_pool(name="outp", bufs=3))
        nob = DIM // sq
        nc.sync.dma_start(out=wo_sb, in_=wo)
        with tc.tile_pool(name="psO", bufs=6, space="PSUM") as psO:
            for st in range(nst):
                # h-inner-over-ob order: each attn stationary tile is loaded
                # once and reused for all nob matmuls.
                pos = [psO.tile([P, sq], F32, tag="po", name=f"po{ob}")
                        for ob in range(nob)]
                for h in range(HL_):
                    for ob in range(nob):
                        nc.tensor.matmul(
                            pos[ob],
                            lhsT=attn_sb[:, h, st * P:(st + 1) * P],
                            rhs=wo_sb[:, h, ob * sq:(ob + 1) * sq],
                            start=(h == 0),
                            stop=(h == HL_ - 1),
                        )
                for ob in range(nob):
                    ot = outp.tile([P, sq], F32, tag="ot")
                    nc.vector.tensor_copy(out=ot, in_=pos[ob])
                    nc.sync.dma_start(
                        out=out_t[st][:, ob * sq:(ob + 1) * sq], in_=ot
                    )

    nc.compile()
    return nc


# ---------------- host side ----------------

def _rope_tables(S):
    inv_freq = 1.0 / (10000.0 ** (np.arange(0, D, 2, dtype=np.float32) / D))
    t = np.arange(S, dtype=np.float32)
    freqs = np.einsum("i,j->ij", t, inv_freq)      # [S, 64]
    cos_h = np.cos(freqs).T                        # [64, S]
    sin_h = np.sin(freqs).T
    cosf = np.concatenate([cos_h, cos_h], 0)       # [128, S]
    sinfs = np.concatenate([-sin_h, sin_h], 0)     # sign-folded
    return cosf, sinfs


def _shard_inputs(x, w_qkv, w_out):
    B, S, DIM = x.shape
    bf = ml_dtypes.bfloat16
    nkt = DIM // P
    cosf, sinfs = _rope_tables(S)
    cosf = cosf.astype(bf)
    sinfs = sinfs.astype(bf)
    in_maps = []
    for c in range(N_CORES):
        b, g = divmod(c, TP)
        h0 = HL * g
        # xt[p, kt, s] = x[b, s, kt*128+p]
        xt = np.ascontiguousarray(
            x[b].reshape(S, nkt, P).transpose(2, 1, 0)
        ).astype(bf)
        # wq[r, p, kt, m] = w_qkv[(h0+r)*128 + m, kt*128 + p]
        wq_s = w_qkv[h0 * D:(h0 + HL) * D]                  # [512, DIM]
        wk_s = w_qkv[HEADS * D + h0 * D:HEADS * D + (h0 + HL) * D]
        wv_s = w_qkv[2 * HEADS * D + h0 * D:2 * HEADS * D + (h0 + HL) * D]
        wq_t = np.ascontiguousarray(
            wq_s.reshape(HL, P, nkt, P).transpose(0, 3, 2, 1)
        ).astype(bf)
        wk_t = np.ascontiguousarray(
            wk_s.reshape(HL, P, nkt, P).transpose(0, 3, 2, 1)
        ).astype(bf)
        # wv[p, kt, vo] = wv_s[vo, kt*128+p]
        wv_t = np.ascontiguousarray(
            wv_s.reshape(HL * D, nkt, P).transpose(2, 1, 0)
        ).astype(bf)
        # wo[p, h, o] = w_out[o, 512g + h*128 + p]
        wo_s = w_out[:, h0 * D:(h0 + HL) * D]               # [DIM, 512]
        wo_t = np.ascontiguousarray(
            wo_s.reshape(DIM, HL, P).transpose(2, 1, 0)
        ).astype(bf)
        in_maps.append(
            {"xt": xt, "wq": wq_t, "wk": wk_t, "wv": wv_t, "wo": wo_t,
             "cosf": cosf, "sinfs": sinfs}
        )
    return in_maps


_NC_CACHE = {}


def _get_nc(S, DIM):
    key = (S, DIM)
    if key not in _NC_CACHE:
        _NC_CACHE[key] = build_kernel(S, DIM)
    return _NC_CACHE[key]


def kernel(x, w_qkv, w_out, trace=False):
    x = np.asarray(x)
    w_qkv = np.asarray(w_qkv)
    w_out = np.asarray(w_out)
    B, S, DIM = x.shape
    nc = _get_nc(S, DIM)
    in_maps = _shard_inputs(x, w_qkv, w_out)
    res = run_bass_kernel_spmd(nc, in_maps, core_ids=list(range(N_CORES)),
                               trace=trace)
    outs = [np.asarray(r["out"], dtype=np.float32) for r in res.results]
    full = np.stack(
        [sum(outs[b * TP:(b + 1) * TP][1:], outs[b * TP]) for b in range(DP)]
    ).astype(np.float32)
    if trace:
        kernel.last_results = res
    return full


# revision 29
# speedup vs baseline: 1.4232x; 1.0420x over previous
"""Multi-head attention (RoPE) Trainium2 kernel, 8 NeuronCores.

Sharding: data-parallel over batch (2) x tensor-parallel over heads (4
heads/core).  Core c handles batch c//4, heads 4*(c%4) .. 4*(c%4)+4.
Each core computes qkv projection for its heads, RoPE, full attention
over its heads, and the out-projection partial (w_out column shard).
The 4 partials per batch are summed on the host (TP all-reduce epilogue
done host-side; no device collective).

Device layouts (per core):
  xt    [128, DIM/128, S]    bf16   x[b].T tiled: xt[p, kt, s] = x[b, s, kt*128+p]
  wq/wk [r, 128, DIM/128, 128] bf16 stationary tiles for transposed proj
  wv    [128, DIM/128, HL*128] bf16 moving tiles for natural v proj
  wo    [128, HL, DIM]       bf16   wo[p, h, o] = w_out[o, 512g + h*128 + p]
  cosf/sinfs [128, S]        bf16   RoPE tables, transposed, halves duplicated,
                                    sin sign-folded (rows 0:64 negated)
  out   [S, DIM]             f32    partial output (natural layout)
"""

import math
import sys
from contextlib import ExitStack

import numpy as np

sys.path.insert(0, "/opt/trn_rl_repo")

import ml_dtypes  # noqa: E402

import concourse.bass as bass  # noqa: E402
import concourse.tile as tile  # noqa: E402
from concourse import bacc, mybir  # noqa: E402
from concourse.bass_utils import run_bass_kernel_spmd  # noqa: E402

P = 128          # partitions / head dim
HEADS = 16
D = 128
N_CORES = 8
DP = 2           # batch shards
TP = 4           # head-group shards
HL = HEADS // TP  # heads per core

BF16 = mybir.dt.bfloat16
F32 = mybir.dt.float32


def build_kernel(S, DIM, HL_=HL, sq=512, sq2=1024, num_devices=N_CORES,
                 debug_taps=False):
    """Build + compile the per-core Bass program (SPMD: same program all cores)."""
    nkt = DIM // P    # contraction tiles for projections
    nst = S // P      # sequence tiles of 128
    nsq = S // sq     # 512-wide free blocks
    sq2 = min(sq2, S)
    nsq2 = S // sq2   # attention s_q groups
    sq_per2 = sq2 // sq
    scale = 1.0 / math.sqrt(D)

    nc = bacc.Bacc("TRN2", debug=False, num_devices=num_devices)

    xt = nc.dram_tensor("xt", [P, nkt, S], BF16, kind="ExternalInput").ap()
    wq = nc.dram_tensor("wq", [HL_, P, nkt, P], BF16, kind="ExternalInput").ap()
    wk = nc.dram_tensor("wk", [HL_, P, nkt, P], BF16, kind="ExternalInput").ap()
    wv = nc.dram_tensor("wv", [P, nkt, HL_ * D], BF16, kind="ExternalInput").ap()
    wo = nc.dram_tensor("wo", [P, HL_, DIM], BF16, kind="ExternalInput").ap()
    cosf = nc.dram_tensor("cosf", [P, S], BF16, kind="ExternalInput").ap()
    sinfs = nc.dram_tensor("sinfs", [P, S], BF16, kind="ExternalInput").ap()
    out = nc.dram_tensor("out", [S, DIM], BF16, kind="ExternalOutput").ap()
    out_t = out.rearrange("(st p) o -> st p o", p=P)
    taps = {}
    if debug_taps:
        for name, shape in (
            ("dbg_q", [P, HL_, S]), ("dbg_k", [P, HL_, S]),
            ("dbg_v", [P, S // P, HL_ * D]), ("dbg_attn", [P, HL_, S]),
        ):
            taps[name] = nc.dram_tensor(
                name, shape, BF16, kind="ExternalOutput").ap()
        for name, shape in (
            ("dbg_ex", [P, sq2]), ("dbg_sm", [1, sq2]),
            ("dbg_av", [P, sq2]), ("dbg_bc", [P, sq2]),
        ):
            taps[name] = nc.dram_tensor(
                name, shape, F32, kind="ExternalOutput").ap()

    with tile.TileContext(nc) as tc, ExitStack() as ctx:
        nc = tc.nc
        # ---- persistent SBUF ----
        big = ctx.enter_context(tc.tile_pool(name="big", bufs=1))
        q_sb = big.tile([P, HL_, S], BF16, tag="q")
        k_sb = big.tile([P, HL_, S], BF16, tag="k")
        v_sb = big.tile([P, nst, HL_ * D], BF16, tag="v")
        attn_sb = big.tile([P, HL_, S], BF16, tag="attn")
        cos_sb = big.tile([P, S], BF16, tag="cos")
        sin_sb = big.tile([P, S], BF16, tag="sin")
        ones_sb = big.tile([P, 1], BF16, tag="ones")
        wv_sb = big.tile([P, nkt, HL_ * D], BF16, tag="wv")
        wo_sb = big.tile([P, HL_, DIM], BF16, tag="wo")

        nc.vector.memset(ones_sb, 1.0)

        xpool = ctx.enter_context(tc.tile_pool(name="xpool", bufs=2))
        wpool = ctx.enter_context(tc.tile_pool(name="wpool", bufs=3))
        rope = ctx.enter_context(tc.tile_pool(name="rope", bufs=2))

        # ================= Phase 1: qkv projection + RoPE =================
        # x streamed in sq-wide chunks; all 12 output row-tiles per chunk.
        spt = sq // P  # s-tiles of 128 per chunk
        with tc.tile_pool(name="psA", bufs=4, space="PSUM") as psA:
            for j in range(nsq):
                win = bass.ds(j * sq, sq)
                xc = xpool.tile([P, nkt, sq], BF16, tag="xc")
                for kt in range(nkt):
                    nc.sync.dma_start(out=xc[:, kt, :], in_=xt[:, kt, win])
                if j == 0:
                    nc.sync.dma_start(out=cos_sb, in_=cosf)
                    nc.sync.dma_start(out=sin_sb, in_=sinfs)
                # q and k (transposed orientation [d, s])
                for which, wdram, dst in (("q", wq, q_sb), ("k", wk, k_sb)):
                    for h in range(HL_):
                        w_t = wpool.tile([P, nkt, P], BF16, tag="w")
                        nc.sync.dma_start(out=w_t, in_=wdram[h])
                        ps = psA.tile([P, sq], F32, tag="ps")
                        for kt in range(nkt):
                            nc.tensor.matmul(
                                ps,
                                lhsT=w_t[:, kt, :],
                                rhs=xc[:, kt, :],
                                start=(kt == 0),
                                stop=(kt == nkt - 1),
                            )
                        # RoPE: dst = p*cos + swap(p)*sin_signed
                        pb = rope.tile([P, sq], BF16, tag="pb")
                        nc.scalar.copy(pb, ps)
                        sw = rope.tile([P, sq], BF16, tag="sw")
                        nc.vector.tensor_copy(out=sw[0:64, :], in_=pb[64:128, :])
                        nc.vector.tensor_copy(out=sw[64:128, :], in_=pb[0:64, :])
                        t1 = rope.tile([P, sq], BF16, tag="t1")
                        nc.vector.tensor_mul(t1, pb, cos_sb[:, win])
                        t2 = rope.tile([P, sq], BF16, tag="t2")
                        nc.vector.tensor_mul(t2, sw, sin_sb[:, win])
                        nc.vector.tensor_add(dst[:, h, win], t1, t2)
                # v (natural orientation [s, d_local])
                if j == 0:
                    nc.sync.dma_start(out=wv_sb, in_=wv)
                for sl in range(spt):
                    st = j * spt + sl
                    ps = psA.tile([P, HL_ * D], F32, tag="ps")
                    for kt in range(nkt):
                        nc.tensor.matmul(
                            ps,
                            lhsT=xc[:, kt, sl * P:(sl + 1) * P],
                            rhs=wv_sb[:, kt, :],
                            start=(kt == 0),
                            stop=(kt == nkt - 1),
                        )
                    nc.scalar.copy(v_sb[:, st, :], ps)

        # ================= Phase 2: attention =================
        expp = ctx.enter_context(tc.tile_pool(name="expp", bufs=6))
        nrm = ctx.enter_context(tc.tile_pool(name="nrm", bufs=1))
        with (
            tc.tile_pool(name="psLG", bufs=2, space="PSUM") as psLG,
            tc.tile_pool(name="psAV", bufs=1, space="PSUM") as psAV,
            tc.tile_pool(name="psSM", bufs=1, space="PSUM") as psSM,
        ):
            PK = min(4, nst)  # ones-matmuls packed per column-tiled group
            for h in range(HL_):
                for j2 in range(nsq2):
                    win2 = bass.ds(j2 * sq2, sq2)
                    av = psAV.tile([P, sq2], F32, tag="av")
                    # sums live in 4 partition rows (0/32/64/96) per c-half:
                    # packed col-tiled ones-matmuls run concurrently on PE.
                    sm = psSM.tile([P, sq2], F32, tag="sm")
                    # software-pipelined: av for tile i-1 issues after
                    # logits for tile i, so PE never queues behind exp.
                    exs = [None] * nst
                    for i in range(nst):
                        lg = psLG.tile([P, sq2], F32, tag="lg")
                        k_tile = k_sb[:, h, i * P:(i + 1) * P]
                        for c in range(sq_per2):
                            nc.tensor.matmul(
                                lg[:, c * sq:(c + 1) * sq],
                                lhsT=k_tile,
                                rhs=q_sb[:, h, bass.ds(j2 * sq2 + c * sq, sq)],
                                start=True,
                                stop=True,
                            )
                        if i > 1:
                            pi = i - 2
                            exp_prev = exs[pi]
                            v_tile = v_sb[:, pi, h * D:(h + 1) * D]
                            for c in range(sq_per2):
                                nc.tensor.matmul(
                                    av[:, bass.ds(c * sq, sq)],
                                    lhsT=v_tile,
                                    rhs=exp_prev[:, bass.ds(c * sq, sq)],
                                    start=(pi == 0),
                                    stop=(pi == nst - 1),
                                )
                        if i > 0 and i % PK == 0:
                            for c in range(sq_per2):
                                cw = bass.ds(c * sq, sq)
                                for r in range(PK):
                                    ii = i - PK + r
                                    nc.tensor.matmul(
                                        sm[32 * r:32 * r + 1, cw],
                                        lhsT=ones_sb,
                                        rhs=exs[ii][:, cw],
                                        start=(ii < PK),
                                        stop=False,
                                        tile_position=(0, 32 * r),
                                    )
                        ex = expp.tile([P, sq2], BF16, tag="ex")
                        nc.scalar.activation(
                            ex, lg, mybir.ActivationFunctionType.Exp, scale=scale
                        )
                        exs[i] = ex
                        if debug_taps and h == 0 and j2 == 0 and i == 0:
                            exf = nrm.tile([P, sq2], F32, tag="dbgex")
                            nc.vector.tensor_copy(out=exf, in_=ex)
                            nc.sync.dma_start(out=taps["dbg_ex"], in_=exf)
                    # tail: last two tiles' av accumulation + final sums pack
                    for pi in (nst - 2, nst - 1):
                        v_tile = v_sb[:, pi, h * D:(h + 1) * D]
                        for c in range(sq_per2):
                            cw = bass.ds(c * sq, sq)
                            nc.tensor.matmul(
                                av[:, cw], lhsT=v_tile, rhs=exs[pi][:, cw],
                                start=(pi == 0), stop=(pi == nst - 1),
                            )
                    for c in range(sq_per2):
                        cw = bass.ds(c * sq, sq)
                        for r in range(PK):
                            ii = nst - PK + r
                            nc.tensor.matmul(
                                sm[32 * r:32 * r + 1, cw],
                                lhsT=ones_sb,
                                rhs=exs[ii][:, cw],
                                start=(ii < PK),
                                stop=True,
                                tile_position=(0, 32 * r),
                            )
                    # free av/sm banks fast (cheap DVE ops), then normalize
                    # off the PE critical path.
                    avf = nrm.tile([P, sq2], F32, tag="avf")
                    nc.vector.tensor_copy(out=avf, in_=av)
                    # combine the PK partial-sum rows into SBUF (DVE can
                    # read at most one PSUM operand per instruction)
                    ssum = nrm.tile([1, sq2], F32, tag="ssum")
                    nc.vector.tensor_copy(out=ssum, in_=sm[0:1, :])
                    for r in range(1, PK):
                        nc.vector.tensor_add(
                            ssum, ssum, sm[32 * r:32 * r + 1, :]
                        )
                    recip = nrm.tile([1, sq2], F32, tag="recip")
                    nc.vector.reciprocal_approx_fast(out=recip, in_=ssum)
                    bcast = nrm.tile([P, sq2], F32, tag="bcast")
                    nc.gpsimd.partition_broadcast(bcast, recip)
                    if debug_taps and h == 0 and j2 == 0:
                        nc.sync.dma_start(out=taps["dbg_sm"], in_=recip)
                        nc.sync.dma_start(out=taps["dbg_av"], in_=avf)
                        nc.sync.dma_start(out=taps["dbg_bc"], in_=bcast)
                    nc.vector.tensor_mul(attn_sb[:, h, win2], avf, bcast)

        if debug_taps:
            nc.sync.dma_start(out=taps["dbg_q"], in_=q_sb)
            nc.sync.dma_start(out=taps["dbg_k"], in_=k_sb)
            nc.sync.dma_start(out=taps["dbg_v"], in_=v_sb)
            nc.sync.dma_start(out=taps["dbg_attn"], in_=attn_sb)

        # ================= Phase 3: out-projection =================
        outp = ctx.enter_context(tc.tile_pool(name="outp", bufs=3))
        nob = DIM // sq
        nc.sync.dma_start(out=wo_sb, in_=wo)
        with tc.tile_pool(name="psO", bufs=6, space="PSUM") as psO:
            for st in range(nst):
                # h-inner-over-ob order: each attn stationary tile is loaded
                # once and reused for all nob matmuls.
                pos = [psO.tile([P, sq], F32, tag="po", name=f"po{ob}")
                        for ob in range(nob)]
                for h in range(HL_):
                    for ob in range(nob):
                        nc.tensor.matmul(
                            pos[ob],
                            lhsT=attn_sb[:, h, st * P:(st + 1) * P],
                            rhs=wo_sb[:, h, ob * sq:(ob + 1) * sq],
                            start=(h == 0),
                            stop=(h == HL_ - 1),
                        )
                for ob in range(nob):
                    ot = outp.tile([P, sq], BF16, tag="ot")
                    nc.vector.tensor_copy(out=ot, in_=pos[ob])
                    nc.sync.dma_start(
                        out=out_t[st][:, ob * sq:(ob + 1) * sq], in_=ot
                    )

    nc.compile()
    return nc


# ---------------- host side ----------------

def _rope_tables(S):
    inv_freq = 1.0 / (10000.0 ** (np.arange(0, D, 2, dtype=np.float32) / D))
    t = np.arange(S, dtype=np.float32)
    freqs = np.einsum("i,j->ij", t, inv_freq)      # [S, 64]
    cos_h = np.cos(freqs).T                        # [64, S]
    sin_h = np.sin(freqs).T
    cosf = np.concatenate([cos_h, cos_h], 0)       # [128, S]
    sinfs = np.concatenate([-sin_h, sin_h], 0)     # sign-folded
    return cosf, sinfs


def _shard_inputs(x, w_qkv, w_out):
    B, S, DIM = x.shape
    bf = ml_dtypes.bfloat16
    nkt = DIM // P
    cosf, sinfs = _rope_tables(S)
    cosf = cosf.astype(bf)
    sinfs = sinfs.astype(bf)
    in_maps = []
    for c in range(N_CORES):
        b, g = divmod(c, TP)
        h0 = HL * g
        # xt[p, kt, s] = x[b, s, kt*128+p]
        xt = np.ascontiguousarray(
            x[b].reshape(S, nkt, P).transpose(2, 1, 0)
        ).astype(bf)
        # wq[r, p, kt, m] = w_qkv[(h0+r)*128 + m, kt*128 + p]
        wq_s = w_qkv[h0 * D:(h0 + HL) * D]                  # [512, DIM]
        wk_s = w_qkv[HEADS * D + h0 * D:HEADS * D + (h0 + HL) * D]
        wv_s = w_qkv[2 * HEADS * D + h0 * D:2 * HEADS * D + (h0 + HL) * D]
        wq_t = np.ascontiguousarray(
            wq_s.reshape(HL, P, nkt, P).transpose(0, 3, 2, 1)
        ).astype(bf)
        wk_t = np.ascontiguousarray(
            wk_s.reshape(HL, P, nkt, P).transpose(0, 3, 2, 1)
        ).astype(bf)
        # wv[p, kt, vo] = wv_s[vo, kt*128+p]
        wv_t = np.ascontiguousarray(
            wv_s.reshape(HL * D, nkt, P).transpose(2, 1, 0)
        ).astype(bf)
        # wo[p, h, o] = w_out[o, 512g + h*128 + p]
        wo_s = w_out[:, h0 * D:(h0 + HL) * D]               # [DIM, 512]
        wo_t = np.ascontiguousarray(
            wo_s.reshape(DIM, HL, P).transpose(2, 1, 0)
        ).astype(bf)
        in_maps.append(
            {"xt": xt, "wq": wq_t, "wk": wk_t, "wv": wv_t, "wo": wo_t,
             "cosf": cosf, "sinfs": sinfs}
        )
    return in_maps


_NC_CACHE = {}


def _get_nc(S, DIM):
    key = (S, DIM)
    if key not in _NC_CACHE:
        _NC_CACHE[key] = build_kernel(S, DIM)
    return _NC_CACHE[key]


def kernel(x, w_qkv, w_out, trace=False):
    x = np.asarray(x)
    w_qkv = np.asarray(w_qkv)
    w_out = np.asarray(w_out)
    B, S, DIM = x.shape
    nc = _get_nc(S, DIM)
    in_maps = _shard_inputs(x, w_qkv, w_out)
    res = run_bass_kernel_spmd(nc, in_maps, core_ids=list(range(N_CORES)),
                               trace=trace)
    outs = [np.asarray(r["out"], dtype=np.float32) for r in res.results]
    full = np.stack(
        [sum(outs[b * TP:(b + 1) * TP][1:], outs[b * TP]) for b in range(DP)]
    ).astype(np.float32)
    if trace:
        kernel.last_results = res
    return full
